# revision 28
# baseline (speedup 1.0000x reference)
"""3-layer GCN + global mean pool + linear head on 8 Trainium2 NeuronCores.

Strategy (dst-sharded message passing, v2):
  - GCN normalization factorizes: norm_e = dinv[src]*dinv[dst], so each conv
    layer is  h' = relu( dinv * ((Adj+I) @ (dinv * h)) @ W + b ).  Only pure
    row gather + segment-sum on device; diagonal scalings are per-node ops.
  - Nodes (and their in-edges, self-loops appended) are sharded across the 8
    cores by contiguous dst ranges.
  - The h~ table is quartered BY BLOCK RANGE (window w = block quarter q of
    every core's slice).  Each quarter is AllGathered separately as soon as
    the previous layer finishes that quarter's blocks, so collectives hide
    under compute.  Gather windows == quarters; int16 indices are relative
    to the window table.
  - Edge stream per core: for sg (7 dst blocks): for w (rotated by sg so the
    first gathers of a layer spread across all four AllGathers): edges of
    the sg's blocks in window w, concatenated unpadded, group padded to 128
    tokens UNIFORMLY across cores (max-core length).  dma_gather fetches
    only the useful 128B halves of the 256B table rows (elem 64 x bf16,
    stride 256B), <=1024 tokens per call, rotated over 4 SWDGE queues.
  - Segment-sum on the TensorEngine: per dst block, for each window span
    chunk, aggT[64f,128d] += msgs[128e,64f].T @ M[128e,128d], M built by a
    broadcast is_equal of edst2 (per-(block,window,chunk) dst_rel or -1)
    against an iota tile.  Chunk spans are uniform across cores (min/max
    over cores); out-of-block tokens carry -1 and contribute zero.
  - The layer weight applies after aggregation, then dinv/bias/relu DVE ops.
  - Mean-pool uses the same one-hot matmul against graph ids; partials are
    AllReduced; the head matmul finishes on every core.
"""
import numpy as np
import ml_dtypes

P = 128
NCORES = 8
NWIN = 4          # block-quarter windows
TMAX = 1024       # max tokens per dma_gather call (SWDGE ring limit)
SGBLK = 14        # dst blocks per super-group (msgs buffer granularity)

# Full-size problem dims (nn_GCN_13881334300836)
N_FULL, E_FULL, D_FULL, C_FULL, G_FULL = 100_000, 1_250_000, 64, 10, 128


# --------------------------------------------------------------------------
# Host preprocessing
# --------------------------------------------------------------------------

def preprocess(x, edge_index, batch, n_cores=NCORES):
    """Shard nodes/edges; build window-rotated, group-padded gather streams
    with cross-core-uniform chunk spans.

    Table row for node n (c=n//npc, local=n-c*npc, p=local%P, b=local//P,
    q=quarter(b)): window q, row (c*P + p)*nbq[q] + (b - b0[q]).
    """
    N, H = x.shape
    assert N % n_cores == 0
    npc = N // n_cores
    nblk = (npc + P - 1) // P
    npad = nblk * P

    # block quarters (windows)
    base, rem = divmod(nblk, NWIN)
    nbq = [base + (1 if q < rem else 0) for q in range(NWIN)]
    b0q = np.cumsum([0] + nbq)          # len NWIN+1
    wrows = [n_cores * P * nbq[q] for q in range(NWIN)]
    assert max(wrows) <= 32768

    sgblk = SGBLK
    if nblk % sgblk != 0:
        sgblk = next((g for g in (7, 8, 6, 5, 4, 9, 10, 3, 2) if nblk % g == 0),
                     nblk)
    nsg = nblk // sgblk

    ei = edge_index.astype(np.int64)
    # degrees include self-loops (PyG gcn_norm), but self-loop messages are
    # local (identity matmul on-device) -- exclude them from the stream
    src_all = ei[0]
    dst_all = ei[1]

    deg = (np.bincount(dst_all, minlength=N) + 1).astype(np.float32)
    dinv = (1.0 / np.sqrt(np.maximum(deg, 1.0))).astype(np.float32)

    # source -> (window, idx16)
    core_of = src_all // npc
    local = src_all - core_of * npc
    p_of = local % P
    b_of = local // P
    q_of = np.searchsorted(b0q, b_of, side="right") - 1
    nbq_arr = np.asarray(nbq)
    idx_of = (core_of * P + p_of) * nbq_arr[q_of] + (b_of - b0q[q_of])

    # per-core sorted edge lists: key (dst block, window), stable
    core_edges = []   # c -> (off, s16, dr)
    cnt_all = np.zeros((n_cores, nblk, NWIN), np.int64)
    for c in range(n_cores):
        lo = c * npc
        m = (dst_all >= lo) & (dst_all < lo + npc)
        s16 = idx_of[m].astype(np.int16)
        w = q_of[m]
        d = dst_all[m] - lo
        db, dr = d // P, (d % P).astype(np.float32)
        key = db * NWIN + w
        order = np.argsort(key, kind="stable")
        s16, dr, key = s16[order], dr[order], key[order]
        cnt = np.bincount(key, minlength=nblk * NWIN).reshape(nblk, NWIN)
        cnt_all[c] = cnt
        off = np.zeros(nblk * NWIN + 1, np.int64)
        np.cumsum(cnt.ravel(), out=off[1:])
        core_edges.append((off, s16, dr))

    # uniform group lengths: for (sg, w): tokens = max_c sum_b cnt, pad to 128
    grp_len = np.zeros((nsg, NWIN), np.int64)
    for sg in range(nsg):
        bs = list(range(sg * sgblk, (sg + 1) * sgblk))
        for w in range(NWIN):
            mx = max(int(cnt_all[c, bs, w].sum()) for c in range(n_cores))
            grp_len[sg, w] = -(-max(mx, 1) // P) * P

    # stream layout: for sg: for w in rot(sg): group
    rot = [[(sg + j) % NWIN for j in range(NWIN)] for sg in range(nsg)]
    grp_tok0 = np.zeros((nsg, NWIN), np.int64)   # token start of (sg, w)
    sg_tok0 = []
    sg_w_ranges = []   # sg -> [(w, tok0, tok1)] in rotated order
    pos = 0
    for sg in range(nsg):
        sg_tok0.append(pos)
        rngs = []
        for w in rot[sg]:
            grp_tok0[sg, w] = pos
            rngs.append((w, pos, pos + int(grp_len[sg, w])))
            pos += int(grp_len[sg, w])
        sg_w_ranges.append(rngs)
    ntok = pos
    nchunk = ntok // P

    # per-(b, w) uniform chunk spans + matmul metadata
    spans = {}          # (b, w) -> (c0, c1)  global chunk ids
    ncol2 = 0
    col2_of = {}        # (b, w) -> starting edst2 column
    for sg in range(nsg):
        for b in range(sg * sgblk, (sg + 1) * sgblk):
            for w in range(NWIN):
                g0 = int(grp_tok0[sg, w])
                t0s, t1s = [], []
                for c in range(n_cores):
                    pre = int(cnt_all[c, sg * sgblk:b, w].sum())
                    cn = int(cnt_all[c, b, w])
                    t0s.append(g0 + pre)
                    t1s.append(g0 + pre + cn)
                c0 = min(t0s) // P
                c1 = -(-max(t1s) // P)
                c1 = max(c1, c0 + 1)
                spans[(b, w)] = (int(c0), int(c1))
                col2_of[(b, w)] = ncol2
                ncol2 += int(c1 - c0)

    # build per-core streams + edst2
    eidx16 = np.zeros((n_cores, 16, ntok // 16), np.int16)
    edst2 = np.full((n_cores, P, ncol2), -1.0, np.float32)
    for c in range(n_cores):
        off, s16, dr = core_edges[c]
        stream = np.zeros(ntok, np.int16)
        drel = np.full(ntok, -1.0, np.float32)   # dst_rel per token
        bof = np.full(ntok, -1, np.int64)        # owning block per token
        for sg in range(nsg):
            for w in range(NWIN):
                t = int(grp_tok0[sg, w])
                for b in range(sg * sgblk, (sg + 1) * sgblk):
                    k = b * NWIN + w
                    sl = slice(off[k], off[k + 1])
                    n_e = int(off[k + 1] - off[k])
                    stream[t:t + n_e] = s16[sl]
                    drel[t:t + n_e] = dr[sl]
                    bof[t:t + n_e] = b
                    t += n_e
        eidx16[c] = stream.reshape(ntok // 16, 16).T
        for (b, w), (c0, c1) in spans.items():
            cw = col2_of[(b, w)]
            seg_d = drel[c0 * P:c1 * P].copy()
            seg_b = bof[c0 * P:c1 * P]
            seg_d[seg_b != b] = -1.0
            edst2[c][:, cw:cw + (c1 - c0)] = seg_d.reshape(c1 - c0, P).T

    dinv_pc = np.zeros((n_cores, P, nblk), np.float32)
    bat_pc = np.full((n_cores, P, nblk), -1.0, np.float32)
    xp_pc = np.zeros((n_cores, P, nblk * H), np.float32)
    xf = np.asarray(x, np.float32)
    for c in range(n_cores):
        dv = np.zeros(npad, np.float32)
        dv[:npc] = dinv[c * npc:(c + 1) * npc]
        dinv_pc[c] = dv.reshape(nblk, P).T
        bt = np.full(npad, -1.0, np.float32)
        bt[:npc] = batch[c * npc:(c + 1) * npc].astype(np.float32)
        bat_pc[c] = bt.reshape(nblk, P).T
        xp = np.zeros((npad, H), np.float32)
        xp[:npc] = xf[c * npc:(c + 1) * npc]
        xp_pc[c] = xp.reshape(nblk, P, H).transpose(1, 0, 2).reshape(P, nblk * H)

    # layer-0 gather tables: h~0 = dinv * x for ALL nodes, window-laid-out
    # (host-computable, so layer 0 needs no AllGather at runtime).
    # fp8e4 rows: [64 B payload | 192 B pad] at the required 256 B stride.
    import concourse.mybir as _mybir
    f8d = _mybir.dt.np(_mybir.dt.float8e4)
    ht0 = (xf * dinv[:, None]).astype(f8d)
    n_all = np.arange(N)
    cn = n_all // npc
    ln = n_all - cn * npc
    pn, bn = ln % P, ln // P
    qn = np.searchsorted(b0q, bn, side="right") - 1
    rn = (cn * P + pn) * nbq_arr[qn] + (bn - b0q[qn])
    x0t = []
    for q in range(NWIN):
        Tq = np.zeros((n_cores * P * nbq[q], 2 * P), f8d)
        mq = qn == q
        Tq[rn[mq], :H] = ht0[n_all[mq]]
        x0t.append(Tq.reshape(n_cores * P, nbq[q] * 2 * P))

    return dict(eidx16=eidx16, edst2=edst2, dinv=dinv_pc, batg=bat_pc,
                xp=xp_pc, x0t=x0t, npc=npc, nblk=nblk, nsg=nsg, sgblk=sgblk,
                ntok=ntok, nchunk=nchunk, ncol2=ncol2, spans=spans,
                col2_of=col2_of, sg_tok0=sg_tok0, sg_w_ranges=sg_w_ranges,
                grp_tok0=grp_tok0, nbq=nbq, b0q=b0q, wrows=wrows, H=H)


# --------------------------------------------------------------------------
# Device kernel builder
# --------------------------------------------------------------------------

def dma_gather_any(gp, out_ap, in_ap, idxs_ap, num_idxs, num_idxs_reg,
                   elem_size, elem_step, single_packet=True, queue_num=0):
    """dma_gather with elem_size_bytes not restricted to %256 (non-transpose
    HBM-source path only; stride (elem_step) must still be a 256B multiple).
    Mirrors bass.GpSimd.dma_gather minus the transpose-only elem assert."""
    import concourse.mybir as mybir
    from concourse import ap_utils
    from concourse.bass import exact_div

    gp._assert_queue_num(queue_num)
    assert idxs_ap.dtype == mybir.dt.int16
    assert in_ap.dtype == out_ap.dtype
    elem_size_bytes = elem_size * mybir.dt.size(in_ap.dtype)
    assert elem_size_bytes > 0
    assert in_ap.space.name == "DRAM"
    assert idxs_ap.space.name == "SBUF"
    assert out_ap.space.name == "SBUF"
    assert ap_utils.ap_is_contiguous(out_ap.ap[1:])
    assert ap_utils.ap_is_contiguous(idxs_ap.ap[1:])
    assert in_ap.ap[-1][1] == out_ap.ap[-1][1] == elem_size
    assert out_ap.ap[0][1] * out_ap.ap[1][1] % 128 == 0
    assert in_ap.ap[0][0] == elem_step
    stride_bytes = elem_step * mybir.dt.size(in_ap.dtype)
    stride_bytes_256 = exact_div(stride_bytes, 256)
    assert stride_bytes_256 < 256
    _in_ap = gp.lower_ap_dma(in_ap, for_custom_bir_dma=True)
    _idxs_ap = gp.lower_ap(idxs_ap)
    _out_ap = gp.lower_ap(out_ap)
    return gp.add_instruction(
        mybir.InstDMAGatherAnt(
            name=gp.bass.get_next_instruction_name(),
            ins=[*_in_ap, _idxs_ap,
                 gp.lower_val_access(gp.to_reg(num_idxs_reg))],
            outs=[_out_ap],
            transpose=False,
            num_idxs=num_idxs,
            elem_size=elem_size,
            stride_bytes_256=stride_bytes_256,
            gen_mode=0,
            single_packet=single_packet,
            queue_num=queue_num,
            sbuf_tokens_per_rank=0,
            sbuf_free_dim_per_rank=0,
            sbuf_free_dim_pad_per_rank=0,
            sbuf_byte_offset=0,
        ))


def build_nc(pp, G, C, n_cores=NCORES, repeat=1, skip=frozenset(),
             nq=4, tmax=TMAX, single_packet=True):
    """Build the Bass program (shared SPMD across n_cores).

    repeat>1 re-runs the whole forward pass that many times inside one NEFF
    (delta-method HW timing only).  skip: timing-experiment knob."""
    import concourse.bacc as bacc
    import concourse.mybir as mybir
    import concourse.tile as tile
    from contextlib import ExitStack

    H = pp["H"]
    nblk, nsg, sgblk = pp["nblk"], pp["nsg"], pp["sgblk"]
    ntok, nchunk, ncol2 = pp["ntok"], pp["nchunk"], pp["ncol2"]
    spans, col2_of = pp["spans"], pp["col2_of"]
    sg_tok0, sg_w_ranges = pp["sg_tok0"], pp["sg_w_ranges"]
    nbq, b0q = pp["nbq"], pp["b0q"]
    RG = [list(range(n_cores))]
    EL = P  # padded table row width in bf16 elements (256B rows)

    f32, bf16 = mybir.dt.float32, mybir.dt.bfloat16
    i16 = mybir.dt.int16
    AL = mybir.AluOpType

    nc = bacc.Bacc("TRN2", target_bir_lowering=False, debug=False,
                   enable_asserts=False, num_devices=n_cores,
                   num_swdge_queues=nq)

    eidx_d = nc.dram_tensor("eidx", [16, ntok // 16], i16, kind="ExternalInput")
    edst_d = nc.dram_tensor("edst", [P, ncol2], bf16, kind="ExternalInput")
    xp_d = nc.dram_tensor("xp", [P, nblk * H], f32, kind="ExternalInput")
    dinv_d = nc.dram_tensor("dinv", [P, nblk], f32, kind="ExternalInput")
    batg_d = nc.dram_tensor("batg", [P, nblk], f32, kind="ExternalInput")
    iota_bf_d = nc.dram_tensor("iota_bf", [P, P], bf16, kind="ExternalInput")
    ident_d = nc.dram_tensor("ident", [P, P], bf16, kind="ExternalInput")
    iota_f_d = nc.dram_tensor("iota_f", [P, P], f32, kind="ExternalInput")
    w_d = [nc.dram_tensor(f"w{l}", [H, H], f32, kind="ExternalInput")
           for l in range(3)]
    bias_d = [nc.dram_tensor(f"bias{l}", [P, H], f32, kind="ExternalInput")
              for l in range(3)]
    wl_d = nc.dram_tensor("wl", [H, C], f32, kind="ExternalInput")
    biasl_d = nc.dram_tensor("biasl", [P, C], f32, kind="ExternalInput")
    cinv_d = nc.dram_tensor("cinv", [P, 1], f32, kind="ExternalInput")
    f8 = mybir.dt.float8e4
    EL8 = 2 * P   # fp8 elems per 256B table row
    x0t_d = [nc.dram_tensor(f"x0t{q}", [n_cores * P, nbq[q] * EL8], f8,
                            kind="ExternalInput") for q in range(NWIN)]
    out_d = nc.dram_tensor("out", [G, C], f32, kind="ExternalOutput")

    with tile.TileContext(nc) as tc:
        with ExitStack() as ctx:
            const = ctx.enter_context(tc.tile_pool(name="const", bufs=1))
            msgs_tp = ctx.enter_context(tc.tile_pool(name="msgs", bufs=2))
            m_tp = ctx.enter_context(tc.tile_pool(name="mb", bufs=3))
            s_tp = ctx.enter_context(tc.tile_pool(name="st", bufs=3))
            e_tp = ctx.enter_context(tc.tile_pool(name="ep", bufs=4))
            agg_ps = ctx.enter_context(tc.tile_pool(name="aggp", bufs=4,
                                                    space="PSUM"))
            out_ps = ctx.enter_context(tc.tile_pool(name="outp", bufs=2,
                                                    space="PSUM"))
            fin_ps = ctx.enter_context(tc.tile_pool(name="finp", bufs=1,
                                                    space="PSUM"))
            dram = ctx.enter_context(tc.tile_pool(name="dram", bufs=1,
                                                  space="DRAM"))

            eidx_sb = const.tile([128, ntok // 16], i16)
            edst_sb = const.tile([P, ncol2], bf16)
            iota_bf = const.tile([P, P], bf16)
            iota_f = const.tile([P, P], f32)
            ident_bf = const.tile([P, P], bf16)
            dinv_sb = const.tile([P, nblk], f32)
            batg_sb = const.tile([P, nblk], f32)
            w_sb = [const.tile([H, H], f32, tag=f"w{l}", name=f"w{l}_sb")
                    for l in range(3)]
            bias_sb = [const.tile([P, H], f32, tag=f"b{l}", name=f"b{l}_sb")
                       for l in range(3)]
            wl_sb = const.tile([H, C], f32)
            biasl_sb = const.tile([P, C], f32)
            cinv_sb = const.tile([P, 1], f32)
            ht_sb = const.tile([P, nblk, EL], bf16)   # h~ slice, 256B rows
            ht8_sb = const.tile([P, nblk, H], f8)     # fp8 h~ (64B payload)
            h3_sb = const.tile([P, nblk * H], f32)
            xp_sb = const.tile([P, nblk * H], f32)

            # idx tile: replicate the [16, S] wrap to all 8 partition groups
            for g8 in range(8):
                nc.sync.dma_start(eidx_sb[:][g8 * 16:(g8 + 1) * 16, :],
                                  eidx_d.ap())
            nc.sync.dma_start(edst_sb[:], edst_d.ap())
            nc.sync.dma_start(iota_bf[:], iota_bf_d.ap())
            nc.sync.dma_start(ident_bf[:], ident_d.ap())
            nc.sync.dma_start(iota_f[:], iota_f_d.ap())
            nc.sync.dma_start(dinv_sb[:], dinv_d.ap())
            nc.sync.dma_start(batg_sb[:], batg_d.ap())
            for l in range(3):
                nc.sync.dma_start(w_sb[l][:], w_d[l].ap())
                nc.sync.dma_start(bias_sb[l][:], bias_d[l].ap())
            nc.sync.dma_start(wl_sb[:], wl_d.ap())
            nc.sync.dma_start(biasl_sb[:], biasl_d.ap())
            nc.sync.dma_start(cinv_sb[:], cinv_d.ap())
            nc.sync.dma_start(xp_sb[:], xp_d.ap())
            # zero the padding feature columns of h~ once
            nc.vector.memset(ht_sb[:], 0.0)
            if "epi" in skip:
                nc.vector.memset(h3_sb[:], 0.0)

            # per-quarter staging + per-(repeat, layer, quarter) shared outs
            in_cc_q = [dram.tile([P, nbq[q] * EL8], f8, tag=f"incc{q}",
                                 name=f"incc{q}") for q in range(NWIN)]
            hfull_rlq = [[[dram.tile([n_cores * P, nbq[q] * EL8], f8,
                                     addr_space="Shared",
                                     tag=f"hf{r}_{l}_{q}",
                                     name=f"hf{r}_{l}_{q}")
                           for q in range(NWIN)] for l in (1, 2)]
                         for r in range(repeat)]
            prd_in = dram.tile([H, P], f32)
            prd_out_r = [dram.tile([H, P], f32, addr_space="Shared",
                                   tag=f"prd_out_{r}", name=f"prd_out_{r}")
                         for r in range(repeat)]

            def issue_ag(r, l, q):
                """Stage quarter q of ht8 (fp8 payload halves of the 256B
                rows) and AllGather it for layer l (hfull index l-1)."""
                nc.sync.dma_start(
                    in_cc_q[q][:].rearrange("p (b e) -> p b e", e=EL8)
                        [:, :, 0:H],
                    ht8_sb[:][:, int(b0q[q]):int(b0q[q + 1]), :])
                if "ag" not in skip:
                    nc.gpsimd.collective_compute(
                        "AllGather", AL.bypass, replica_groups=RG,
                        ins=[in_cc_q[q].opt()],
                        outs=[hfull_rlq[r][l - 1][q].opt()])

            qend = {int(b0q[q + 1]) - 1: q for q in range(NWIN)}

            for _rep in range(repeat):
              hfull = hfull_rlq[_rep]
              prd_out = prd_out_r[_rep]

              # layer-1 input: h~ = dinv * x (bf16) -- self-loop source only;
              # layer 0's gather tables are host inputs (no AllGather)
              for bi in range(nblk):
                if "hscale" not in skip:
                    nc.vector.tensor_scalar(
                        out=ht_sb[:][:, bi, 0:H],
                        in0=xp_sb[:][:, bi * H:(bi + 1) * H],
                        scalar1=dinv_sb[:][:, bi:bi + 1], scalar2=None,
                        op0=AL.mult)

              poolT = fin_ps.tile([H, P], f32, tag="poolT")
              for l in range(3):
                last = l == 2
                if l == 0:
                    gat = [x0t_d[q].ap()
                               .rearrange("p (b e) -> (p b) e", e=EL8)
                           for q in range(NWIN)]
                else:
                    gat = [hfull[l - 1][q][:]
                               .rearrange("p (b e) -> (p b) e", e=EL8)
                           for q in range(NWIN)]

                call_no = 0
                for sg in range(nsg):
                    tok0 = sg_tok0[sg]
                    sg_ntok = sg_w_ranges[sg][-1][2] - tok0
                    msgs = msgs_tp.tile([P, sg_ntok // P, H], f8,
                                        tag="msgs", name="msgs")
                    for (ww, t0, t1) in sg_w_ranges[sg]:
                        t = t0
                        while t < t1:
                            tc_ = min(tmax, t1 - t)
                            if "gather" in skip:
                                t += tc_
                                continue
                            dma_gather_any(
                                nc.gpsimd,
                                out_ap=msgs[:][:, (t - tok0) // P:
                                               (t - tok0 + tc_) // P, :],
                                in_ap=gat[ww][:, 0:H],
                                idxs_ap=eidx_sb[:][:, t // 16:(t + tc_) // 16],
                                num_idxs=tc_, num_idxs_reg=tc_,
                                elem_size=H, elem_step=EL8,
                                single_packet=single_packet,
                                queue_num=call_no % nq)
                            call_no += 1
                            t += tc_
                    for bi in range(sg * sgblk, (sg + 1) * sgblk):
                        aggT = agg_ps.tile([H, P], f32, tag="agg", name="agg")
                        kbt = sum(spans[(bi, w)][1] - spans[(bi, w)][0]
                                  for w in range(NWIN))
                        nmm = 1 + kbt
                        if "mm" not in skip:
                            # self-loop: aggT[f, d] += ht[d, b, f]
                            nc.tensor.matmul(
                                aggT[:], lhsT=ht_sb[:][:, bi, 0:H],
                                rhs=ident_bf[:], start=True, stop=False)
                        imm = 1
                        cw0 = col2_of[(bi, 0)]
                        MB = m_tp.tile([P, kbt * P], f8, tag="MB",
                                       name="MB")
                        if "mb" not in skip:
                            nc.vector.tensor_tensor(
                                out=MB[:].rearrange("p (c q) -> p c q", q=P),
                                in0=edst_sb[:][:, cw0:cw0 + kbt]
                                    .to_broadcast([P, kbt, P]),
                                in1=iota_bf[:][:, None, :]
                                    .to_broadcast([P, kbt, P]),
                                op=AL.is_equal)
                        MBr = MB[:].rearrange("p (c q) -> p c q", q=P)
                        for w in range(NWIN):
                            c0, c1 = spans[(bi, w)]
                            kb = c1 - c0
                            joff = col2_of[(bi, w)] - cw0
                            j = 0
                            while j < kb:
                                mc = c0 + j - tok0 // P
                                pair = j + 1 < kb
                                nj = 2 if pair else 1
                                if "mm" in skip:
                                    imm += nj
                                    j += nj
                                    continue
                                if pair:
                                    nc.tensor.matmul(
                                        aggT[:],
                                        lhsT=msgs[:][:, mc:mc + 2, :],
                                        rhs=MBr[:, joff + j:joff + j + 2, :],
                                        start=(imm == 0),
                                        stop=(imm + 2 == nmm),
                                        perf_mode=(
                                            mybir.MatmulPerfMode.DoubleRow),
                                    )
                                else:
                                    nc.tensor.matmul(
                                        aggT[:],
                                        lhsT=msgs[:][:, mc, :],
                                        rhs=MBr[:, joff + j, :],
                                        start=(imm == 0),
                                        stop=(imm + 1 == nmm))
                                imm += nj
                                j += nj
                        if "epi" in skip:
                            continue
                        sT = s_tp.tile([H, P], f32, tag="sT", name="sT")
                        nc.scalar.copy(out=sT[:], in_=aggT[:])
                        outb = out_ps.tile([P, H], f32, tag="outb",
                                           name="outb")
                        nc.tensor.matmul(outb[:], lhsT=sT[:], rhs=w_sb[l][:],
                                         start=True, stop=True)
                        dcol = dinv_sb[:][:, bi:bi + 1]
                        t1_ = e_tp.tile([P, H], f32, tag="t1", name="t1")
                        nc.vector.tensor_scalar(
                            out=t1_[:], in0=outb[:], scalar1=dcol,
                            scalar2=None, op0=AL.mult)
                        if not last:
                            t2 = e_tp.tile([P, H], f32, tag="t2", name="t2")
                            nc.vector.tensor_tensor(
                                out=t2[:], in0=t1_[:], in1=bias_sb[l][:],
                                op=AL.add)
                            nc.vector.tensor_scalar(
                                out=ht_sb[:][:, bi, 0:H], in0=t2[:],
                                scalar1=0.0, scalar2=dcol,
                                op0=AL.max, op1=AL.mult)
                            nc.vector.tensor_scalar(
                                out=ht8_sb[:][:, bi, :], in0=t2[:],
                                scalar1=0.0, scalar2=dcol,
                                op0=AL.max, op1=AL.mult)
                            if bi in qend:
                                issue_ag(_rep, l + 1, qend[bi])
                        else:
                            nc.vector.tensor_tensor(
                                out=h3_sb[:][:, bi * H:(bi + 1) * H],
                                in0=t1_[:], in1=bias_sb[l][:], op=AL.add)

              # pooling: poolT[f, g] = sum_n h3[n, f] * (batch[n] == g)
              for bi in range(nblk):
                Mg = m_tp.tile([P, P], f32, tag="Mg", name="Mg")
                nc.vector.tensor_scalar(
                    out=Mg[:], in0=iota_f[:],
                    scalar1=batg_sb[:][:, bi:bi + 1], scalar2=None,
                    op0=AL.is_equal)
                nc.tensor.matmul(poolT[:],
                                 lhsT=h3_sb[:][:, bi * H:(bi + 1) * H],
                                 rhs=Mg[:], start=(bi == 0),
                                 stop=(bi == nblk - 1))
              poolT_sb = s_tp.tile([H, P], f32, tag="poolTs")
              nc.vector.tensor_copy(out=poolT_sb[:], in_=poolT[:])
              nc.sync.dma_start(prd_in[:], poolT_sb[:])
              nc.gpsimd.collective_compute(
                  "AllReduce", AL.add, replica_groups=RG,
                  ins=[prd_in.opt()], outs=[prd_out.opt()])
              poolF = s_tp.tile([H, P], f32, tag="poolF")
              nc.sync.dma_start(poolF[:], prd_out[:])
              fin = fin_ps.tile([P, C], f32, tag="fin")
              nc.tensor.matmul(fin[:], lhsT=poolF[:], rhs=wl_sb[:],
                               start=True, stop=True)
              outf = e_tp.tile([P, C], f32, tag="outf")
              nc.vector.tensor_scalar(out=outf[:], in0=fin[:],
                                      scalar1=cinv_sb[:], scalar2=None,
                                      op0=AL.mult)
              outf2 = e_tp.tile([P, C], f32, tag="outf2")
              nc.vector.tensor_tensor(out=outf2[:], in0=outf[:],
                                      in1=biasl_sb[:], op=AL.add)
              nc.sync.dma_start(out_d.ap()[:, :], outf2[:][:G, :])

    nc.compile()
    return nc


def make_in_maps(pp, weights, G, n_cores=NCORES):
    W1, b1, W2, b2, W3, b3, Wl, bl, counts = weights
    H = pp["H"]
    C = np.asarray(Wl).shape[1]
    bf = ml_dtypes.bfloat16
    iota_row = np.arange(P, dtype=np.float32)
    iota_bf = np.ascontiguousarray(np.broadcast_to(iota_row, (P, P))).astype(bf)
    iota_f = np.ascontiguousarray(np.broadcast_to(iota_row, (P, P)))
    cinv = np.ones((P, 1), np.float32)
    cinv[:G, 0] = 1.0 / np.maximum(counts, 1.0)
    shared = {
        "iota_bf": iota_bf, "iota_f": iota_f,
        "ident": np.eye(P, dtype=np.float32).astype(bf),
        **{f"x0t{q}": pp["x0t"][q] for q in range(NWIN)},
        "w0": np.asarray(W1, np.float32), "w1": np.asarray(W2, np.float32),
        "w2": np.asarray(W3, np.float32),
        "bias0": np.ascontiguousarray(np.broadcast_to(b1, (P, H))).astype(np.float32),
        "bias1": np.ascontiguousarray(np.broadcast_to(b2, (P, H))).astype(np.float32),
        "bias2": np.ascontiguousarray(np.broadcast_to(b3, (P, H))).astype(np.float32),
        "wl": np.asarray(Wl, np.float32),
        "biasl": np.ascontiguousarray(np.broadcast_to(bl, (P, C))).astype(np.float32),
        "cinv": cinv,
    }
    maps = []
    for c in range(n_cores):
        m = dict(shared)
        m["eidx"] = pp["eidx16"][c]
        m["edst"] = pp["edst2"][c].astype(bf)
        m["xp"] = pp["xp"][c]
        m["dinv"] = pp["dinv"][c]
        m["batg"] = pp["batg"][c]
        maps.append(m)
    return maps


LAST_RESULT = None
LAST_NC = None
LAST_IN_MAPS = None
LAST_BUILD = None


def kernel(x, edge_index, batch, W1, b1, W2, b2, W3, b3, Wl, bl, **run_kwargs):
    """Full-input entry point. Shards across 8 cores, runs on HW, gathers."""
    global LAST_RESULT, LAST_NC, LAST_IN_MAPS, LAST_BUILD
    from concourse.bass_utils import run_bass_kernel_spmd

    x = np.asarray(x, np.float32)
    edge_index = np.asarray(edge_index)
    batch = np.asarray(batch)
    G = G_FULL
    C = np.asarray(Wl).shape[1]

    pp = preprocess(x, edge_index, batch)
    counts = np.bincount(batch.astype(np.int64), minlength=G).astype(np.float32)
    nc = build_nc(pp, G, C)
    in_maps = make_in_maps(pp, (W1, b1, W2, b2, W3, b3, Wl, bl, counts), G)
    res = run_bass_kernel_spmd(nc, in_maps, core_ids=list(range(NCORES)),
                               **run_kwargs)
    LAST_RESULT, LAST_NC, LAST_IN_MAPS = res, nc, in_maps
    LAST_BUILD = dict(pp=pp, G=G, C=C)
    return res.results[0]["out"].astype(np.float32)


# revision 29
# speedup vs baseline: 1.0483x; 1.0483x over previous
"""3-layer GCN + global mean pool + linear head on 8 Trainium2 NeuronCores.

Strategy (dst-sharded message passing, v2):
  - GCN normalization factorizes: norm_e = dinv[src]*dinv[dst], so each conv
    layer is  h' = relu( dinv * ((Adj+I) @ (dinv * h)) @ W + b ).  Only pure
    row gather + segment-sum on device; diagonal scalings are per-node ops.
  - Nodes (and their in-edges, self-loops appended) are sharded across the 8
    cores by contiguous dst ranges.
  - The h~ table is quartered BY BLOCK RANGE (window w = block quarter q of
    every core's slice).  Each quarter is AllGathered separately as soon as
    the previous layer finishes that quarter's blocks, so collectives hide
    under compute.  Gather windows == quarters; int16 indices are relative
    to the window table.
  - Edge stream per core: for sg (7 dst blocks): for w (rotated by sg so the
    first gathers of a layer spread across all four AllGathers): edges of
    the sg's blocks in window w, concatenated unpadded, group padded to 128
    tokens UNIFORMLY across cores (max-core length).  dma_gather fetches
    only the useful 128B halves of the 256B table rows (elem 64 x bf16,
    stride 256B), <=1024 tokens per call, rotated over 4 SWDGE queues.
  - Segment-sum on the TensorEngine: per dst block, for each window span
    chunk, aggT[64f,128d] += msgs[128e,64f].T @ M[128e,128d], M built by a
    broadcast is_equal of edst2 (per-(block,window,chunk) dst_rel or -1)
    against an iota tile.  Chunk spans are uniform across cores (min/max
    over cores); out-of-block tokens carry -1 and contribute zero.
  - The layer weight applies after aggregation, then dinv/bias/relu DVE ops.
  - Mean-pool uses the same one-hot matmul against graph ids; partials are
    AllReduced; the head matmul finishes on every core.
"""
import numpy as np
import ml_dtypes

P = 128
NCORES = 8
NWIN = 4          # block-quarter windows
TMAX = 1024       # max tokens per dma_gather call (SWDGE ring limit)
SGBLK = 7         # dst blocks per super-group (msgs buffer granularity)

# Full-size problem dims (nn_GCN_13881334300836)
N_FULL, E_FULL, D_FULL, C_FULL, G_FULL = 100_000, 1_250_000, 64, 10, 128


# --------------------------------------------------------------------------
# Host preprocessing
# --------------------------------------------------------------------------

def preprocess(x, edge_index, batch, n_cores=NCORES):
    """Shard nodes/edges; build window-rotated, group-padded gather streams
    with cross-core-uniform chunk spans.

    Table row for node n (c=n//npc, local=n-c*npc, p=local%P, b=local//P,
    q=quarter(b)): window q, row (c*P + p)*nbq[q] + (b - b0[q]).
    """
    N, H = x.shape
    assert N % n_cores == 0
    npc = N // n_cores
    nblk = (npc + P - 1) // P
    npad = nblk * P

    # block quarters (windows)
    base, rem = divmod(nblk, NWIN)
    nbq = [base + (1 if q < rem else 0) for q in range(NWIN)]
    b0q = np.cumsum([0] + nbq)          # len NWIN+1
    wrows = [n_cores * P * nbq[q] for q in range(NWIN)]
    assert max(wrows) <= 32768

    sgblk = SGBLK
    if nblk % sgblk != 0:
        sgblk = next((g for g in (7, 8, 6, 5, 4, 9, 10, 3, 2) if nblk % g == 0),
                     nblk)
    nsg = nblk // sgblk

    ei = edge_index.astype(np.int64)
    # degrees include self-loops (PyG gcn_norm), but self-loop messages are
    # local (identity matmul on-device) -- exclude them from the stream
    src_all = ei[0]
    dst_all = ei[1]

    deg = (np.bincount(dst_all, minlength=N) + 1).astype(np.float32)
    dinv = (1.0 / np.sqrt(np.maximum(deg, 1.0))).astype(np.float32)

    # source -> (window, idx16)
    core_of = src_all // npc
    local = src_all - core_of * npc
    p_of = local % P
    b_of = local // P
    q_of = np.searchsorted(b0q, b_of, side="right") - 1
    nbq_arr = np.asarray(nbq)
    idx_of = (core_of * P + p_of) * nbq_arr[q_of] + (b_of - b0q[q_of])

    # per-core sorted edge lists: key (dst block, window), stable
    core_edges = []   # c -> (off, s16, dr)
    cnt_all = np.zeros((n_cores, nblk, NWIN), np.int64)
    for c in range(n_cores):
        lo = c * npc
        m = (dst_all >= lo) & (dst_all < lo + npc)
        s16 = idx_of[m].astype(np.int16)
        w = q_of[m]
        d = dst_all[m] - lo
        db, dr = d // P, (d % P).astype(np.float32)
        key = db * NWIN + w
        order = np.argsort(key, kind="stable")
        s16, dr, key = s16[order], dr[order], key[order]
        cnt = np.bincount(key, minlength=nblk * NWIN).reshape(nblk, NWIN)
        cnt_all[c] = cnt
        off = np.zeros(nblk * NWIN + 1, np.int64)
        np.cumsum(cnt.ravel(), out=off[1:])
        core_edges.append((off, s16, dr))

    # uniform group lengths: for (sg, w): tokens = max_c sum_b cnt, pad to 128
    grp_len = np.zeros((nsg, NWIN), np.int64)
    for sg in range(nsg):
        bs = list(range(sg * sgblk, (sg + 1) * sgblk))
        for w in range(NWIN):
            mx = max(int(cnt_all[c, bs, w].sum()) for c in range(n_cores))
            grp_len[sg, w] = -(-max(mx, 1) // P) * P

    # stream layout: for sg: for w in rot(sg): group
    rot = [[(sg + j) % NWIN for j in range(NWIN)] for sg in range(nsg)]
    grp_tok0 = np.zeros((nsg, NWIN), np.int64)   # token start of (sg, w)
    sg_tok0 = []
    sg_w_ranges = []   # sg -> [(w, tok0, tok1)] in rotated order
    pos = 0
    for sg in range(nsg):
        sg_tok0.append(pos)
        rngs = []
        for w in rot[sg]:
            grp_tok0[sg, w] = pos
            rngs.append((w, pos, pos + int(grp_len[sg, w])))
            pos += int(grp_len[sg, w])
        sg_w_ranges.append(rngs)
    ntok = pos
    nchunk = ntok // P

    # per-(b, w) uniform chunk spans + matmul metadata
    spans = {}          # (b, w) -> (c0, c1)  global chunk ids
    ncol2 = 0
    col2_of = {}        # (b, w) -> starting edst2 column
    for sg in range(nsg):
        for b in range(sg * sgblk, (sg + 1) * sgblk):
            for w in range(NWIN):
                g0 = int(grp_tok0[sg, w])
                t0s, t1s = [], []
                for c in range(n_cores):
                    pre = int(cnt_all[c, sg * sgblk:b, w].sum())
                    cn = int(cnt_all[c, b, w])
                    t0s.append(g0 + pre)
                    t1s.append(g0 + pre + cn)
                c0 = min(t0s) // P
                c1 = -(-max(t1s) // P)
                c1 = max(c1, c0 + 1)
                spans[(b, w)] = (int(c0), int(c1))
                col2_of[(b, w)] = ncol2
                ncol2 += int(c1 - c0)

    # build per-core streams + edst2
    eidx16 = np.zeros((n_cores, 16, ntok // 16), np.int16)
    edst2 = np.full((n_cores, P, ncol2), -1.0, np.float32)
    for c in range(n_cores):
        off, s16, dr = core_edges[c]
        stream = np.zeros(ntok, np.int16)
        drel = np.full(ntok, -1.0, np.float32)   # dst_rel per token
        bof = np.full(ntok, -1, np.int64)        # owning block per token
        for sg in range(nsg):
            for w in range(NWIN):
                t = int(grp_tok0[sg, w])
                for b in range(sg * sgblk, (sg + 1) * sgblk):
                    k = b * NWIN + w
                    sl = slice(off[k], off[k + 1])
                    n_e = int(off[k + 1] - off[k])
                    stream[t:t + n_e] = s16[sl]
                    drel[t:t + n_e] = dr[sl]
                    bof[t:t + n_e] = b
                    t += n_e
        eidx16[c] = stream.reshape(ntok // 16, 16).T
        for (b, w), (c0, c1) in spans.items():
            cw = col2_of[(b, w)]
            seg_d = drel[c0 * P:c1 * P].copy()
            seg_b = bof[c0 * P:c1 * P]
            seg_d[seg_b != b] = -1.0
            edst2[c][:, cw:cw + (c1 - c0)] = seg_d.reshape(c1 - c0, P).T

    dinv_pc = np.zeros((n_cores, P, nblk), np.float32)
    bat_pc = np.full((n_cores, P, nblk), -1.0, np.float32)
    xp_pc = np.zeros((n_cores, P, nblk * H), np.float32)
    xf = np.asarray(x, np.float32)
    for c in range(n_cores):
        dv = np.zeros(npad, np.float32)
        dv[:npc] = dinv[c * npc:(c + 1) * npc]
        dinv_pc[c] = dv.reshape(nblk, P).T
        bt = np.full(npad, -1.0, np.float32)
        bt[:npc] = batch[c * npc:(c + 1) * npc].astype(np.float32)
        bat_pc[c] = bt.reshape(nblk, P).T
        xp = np.zeros((npad, H), np.float32)
        xp[:npc] = xf[c * npc:(c + 1) * npc]
        xp_pc[c] = xp.reshape(nblk, P, H).transpose(1, 0, 2).reshape(P, nblk * H)

    # layer-0 gather tables: h~0 = dinv * x for ALL nodes, window-laid-out
    # (host-computable, so layer 0 needs no AllGather at runtime).
    # fp8e4 rows: [64 B payload | 192 B pad] at the required 256 B stride.
    import concourse.mybir as _mybir
    f8d = _mybir.dt.np(_mybir.dt.float8e4)
    ht0 = (xf * dinv[:, None]).astype(f8d)
    n_all = np.arange(N)
    cn = n_all // npc
    ln = n_all - cn * npc
    pn, bn = ln % P, ln // P
    qn = np.searchsorted(b0q, bn, side="right") - 1
    rn = (cn * P + pn) * nbq_arr[qn] + (bn - b0q[qn])
    x0t = []
    for q in range(NWIN):
        Tq = np.zeros((n_cores * P * nbq[q], 2 * P), f8d)
        mq = qn == q
        Tq[rn[mq], :H] = ht0[n_all[mq]]
        x0t.append(Tq.reshape(n_cores * P, nbq[q] * 2 * P))

    return dict(eidx16=eidx16, edst2=edst2, dinv=dinv_pc, batg=bat_pc,
                xp=xp_pc, x0t=x0t, npc=npc, nblk=nblk, nsg=nsg, sgblk=sgblk,
                ntok=ntok, nchunk=nchunk, ncol2=ncol2, spans=spans,
                col2_of=col2_of, sg_tok0=sg_tok0, sg_w_ranges=sg_w_ranges,
                grp_tok0=grp_tok0, nbq=nbq, b0q=b0q, wrows=wrows, H=H)


# --------------------------------------------------------------------------
# Device kernel builder
# --------------------------------------------------------------------------

def dma_gather_any(gp, out_ap, in_ap, idxs_ap, num_idxs, num_idxs_reg,
                   elem_size, elem_step, single_packet=True, queue_num=0):
    """dma_gather with elem_size_bytes not restricted to %256 (non-transpose
    HBM-source path only; stride (elem_step) must still be a 256B multiple).
    Mirrors bass.GpSimd.dma_gather minus the transpose-only elem assert."""
    import concourse.mybir as mybir
    from concourse import ap_utils
    from concourse.bass import exact_div

    gp._assert_queue_num(queue_num)
    assert idxs_ap.dtype == mybir.dt.int16
    assert in_ap.dtype == out_ap.dtype
    elem_size_bytes = elem_size * mybir.dt.size(in_ap.dtype)
    assert elem_size_bytes > 0
    assert in_ap.space.name == "DRAM"
    assert idxs_ap.space.name == "SBUF"
    assert out_ap.space.name == "SBUF"
    assert ap_utils.ap_is_contiguous(out_ap.ap[1:])
    assert ap_utils.ap_is_contiguous(idxs_ap.ap[1:])
    assert in_ap.ap[-1][1] == out_ap.ap[-1][1] == elem_size
    assert out_ap.ap[0][1] * out_ap.ap[1][1] % 128 == 0
    assert in_ap.ap[0][0] == elem_step
    stride_bytes = elem_step * mybir.dt.size(in_ap.dtype)
    stride_bytes_256 = exact_div(stride_bytes, 256)
    assert stride_bytes_256 < 256
    _in_ap = gp.lower_ap_dma(in_ap, for_custom_bir_dma=True)
    _idxs_ap = gp.lower_ap(idxs_ap)
    _out_ap = gp.lower_ap(out_ap)
    return gp.add_instruction(
        mybir.InstDMAGatherAnt(
            name=gp.bass.get_next_instruction_name(),
            ins=[*_in_ap, _idxs_ap,
                 gp.lower_val_access(gp.to_reg(num_idxs_reg))],
            outs=[_out_ap],
            transpose=False,
            num_idxs=num_idxs,
            elem_size=elem_size,
            stride_bytes_256=stride_bytes_256,
            gen_mode=0,
            single_packet=single_packet,
            queue_num=queue_num,
            sbuf_tokens_per_rank=0,
            sbuf_free_dim_per_rank=0,
            sbuf_free_dim_pad_per_rank=0,
            sbuf_byte_offset=0,
        ))


def build_nc(pp, G, C, n_cores=NCORES, repeat=1, skip=frozenset(),
             nq=4, tmax=TMAX, single_packet=True):
    """Build the Bass program (shared SPMD across n_cores).

    repeat>1 re-runs the whole forward pass that many times inside one NEFF
    (delta-method HW timing only).  skip: timing-experiment knob."""
    import concourse.bacc as bacc
    import concourse.mybir as mybir
    import concourse.tile as tile
    from contextlib import ExitStack

    H = pp["H"]
    nblk, nsg, sgblk = pp["nblk"], pp["nsg"], pp["sgblk"]
    ntok, nchunk, ncol2 = pp["ntok"], pp["nchunk"], pp["ncol2"]
    spans, col2_of = pp["spans"], pp["col2_of"]
    sg_tok0, sg_w_ranges = pp["sg_tok0"], pp["sg_w_ranges"]
    nbq, b0q = pp["nbq"], pp["b0q"]
    RG = [list(range(n_cores))]
    EL = P  # padded table row width in bf16 elements (256B rows)

    f32, bf16 = mybir.dt.float32, mybir.dt.bfloat16
    i16 = mybir.dt.int16
    AL = mybir.AluOpType

    nc = bacc.Bacc("TRN2", target_bir_lowering=False, debug=False,
                   enable_asserts=False, num_devices=n_cores,
                   num_swdge_queues=nq)

    eidx_d = nc.dram_tensor("eidx", [16, ntok // 16], i16, kind="ExternalInput")
    edst_d = nc.dram_tensor("edst", [P, ncol2], bf16, kind="ExternalInput")
    xp_d = nc.dram_tensor("xp", [P, nblk * H], f32, kind="ExternalInput")
    dinv_d = nc.dram_tensor("dinv", [P, nblk], f32, kind="ExternalInput")
    batg_d = nc.dram_tensor("batg", [P, nblk], f32, kind="ExternalInput")
    iota_bf_d = nc.dram_tensor("iota_bf", [P, P], bf16, kind="ExternalInput")
    ident_d = nc.dram_tensor("ident", [P, P], bf16, kind="ExternalInput")
    iota_f_d = nc.dram_tensor("iota_f", [P, P], f32, kind="ExternalInput")
    w_d = [nc.dram_tensor(f"w{l}", [H, H], f32, kind="ExternalInput")
           for l in range(3)]
    bias_d = [nc.dram_tensor(f"bias{l}", [P, H], f32, kind="ExternalInput")
              for l in range(3)]
    wl_d = nc.dram_tensor("wl", [H, C], f32, kind="ExternalInput")
    biasl_d = nc.dram_tensor("biasl", [P, C], f32, kind="ExternalInput")
    cinv_d = nc.dram_tensor("cinv", [P, 1], f32, kind="ExternalInput")
    f8 = mybir.dt.float8e4
    EL8 = 2 * P   # fp8 elems per 256B table row
    x0t_d = [nc.dram_tensor(f"x0t{q}", [n_cores * P, nbq[q] * EL8], f8,
                            kind="ExternalInput") for q in range(NWIN)]
    out_d = nc.dram_tensor("out", [G, C], f32, kind="ExternalOutput")

    with tile.TileContext(nc) as tc:
        with ExitStack() as ctx:
            const = ctx.enter_context(tc.tile_pool(name="const", bufs=1))
            msgs_tp = ctx.enter_context(tc.tile_pool(name="msgs", bufs=2))
            m_tp = ctx.enter_context(tc.tile_pool(name="mb", bufs=3))
            s_tp = ctx.enter_context(tc.tile_pool(name="st", bufs=3))
            e_tp = ctx.enter_context(tc.tile_pool(name="ep", bufs=4))
            agg_ps = ctx.enter_context(tc.tile_pool(name="aggp", bufs=4,
                                                    space="PSUM"))
            out_ps = ctx.enter_context(tc.tile_pool(name="outp", bufs=2,
                                                    space="PSUM"))
            fin_ps = ctx.enter_context(tc.tile_pool(name="finp", bufs=1,
                                                    space="PSUM"))
            dram = ctx.enter_context(tc.tile_pool(name="dram", bufs=1,
                                                  space="DRAM"))

            eidx_sb = const.tile([128, ntok // 16], i16)
            edst_sb = const.tile([P, ncol2], bf16)
            iota_bf = const.tile([P, P], bf16)
            iota_f = const.tile([P, P], f32)
            ident_bf = const.tile([P, P], bf16)
            dinv_sb = const.tile([P, nblk], f32)
            batg_sb = const.tile([P, nblk], f32)
            w_sb = [const.tile([H, H], f32, tag=f"w{l}", name=f"w{l}_sb")
                    for l in range(3)]
            bias_sb = [const.tile([P, H], f32, tag=f"b{l}", name=f"b{l}_sb")
                       for l in range(3)]
            wl_sb = const.tile([H, C], f32)
            biasl_sb = const.tile([P, C], f32)
            cinv_sb = const.tile([P, 1], f32)
            ht_sb = const.tile([P, nblk, EL], bf16)   # h~ slice, 256B rows
            ht8_sb = const.tile([P, nblk, H], f8)     # fp8 h~ (64B payload)
            h3_sb = const.tile([P, nblk * H], f32)
            xp_sb = const.tile([P, nblk * H], f32)

            # idx tile: replicate the [16, S] wrap to all 8 partition groups
            for g8 in range(8):
                nc.sync.dma_start(eidx_sb[:][g8 * 16:(g8 + 1) * 16, :],
                                  eidx_d.ap())
            nc.sync.dma_start(edst_sb[:], edst_d.ap())
            nc.sync.dma_start(iota_bf[:], iota_bf_d.ap())
            nc.sync.dma_start(ident_bf[:], ident_d.ap())
            nc.sync.dma_start(iota_f[:], iota_f_d.ap())
            nc.sync.dma_start(dinv_sb[:], dinv_d.ap())
            nc.sync.dma_start(batg_sb[:], batg_d.ap())
            for l in range(3):
                nc.sync.dma_start(w_sb[l][:], w_d[l].ap())
                nc.sync.dma_start(bias_sb[l][:], bias_d[l].ap())
            nc.sync.dma_start(wl_sb[:], wl_d.ap())
            nc.sync.dma_start(biasl_sb[:], biasl_d.ap())
            nc.sync.dma_start(cinv_sb[:], cinv_d.ap())
            nc.sync.dma_start(xp_sb[:], xp_d.ap())
            # zero the padding feature columns of h~ once
            nc.vector.memset(ht_sb[:], 0.0)
            if "epi" in skip:
                nc.vector.memset(h3_sb[:], 0.0)

            # per-quarter staging + per-(repeat, layer, quarter) shared outs
            in_cc_q = [dram.tile([P, nbq[q] * EL8], f8, tag=f"incc{q}",
                                 name=f"incc{q}") for q in range(NWIN)]
            hfull_rlq = [[[dram.tile([n_cores * P, nbq[q] * EL8], f8,
                                     addr_space="Shared",
                                     tag=f"hf{r}_{l}_{q}",
                                     name=f"hf{r}_{l}_{q}")
                           for q in range(NWIN)] for l in (1, 2)]
                         for r in range(repeat)]
            prd_in = dram.tile([H, P], f32)
            prd_out_r = [dram.tile([H, P], f32, addr_space="Shared",
                                   tag=f"prd_out_{r}", name=f"prd_out_{r}")
                         for r in range(repeat)]

            def issue_ag(r, l, q):
                """Stage quarter q of ht8 (fp8 payload halves of the 256B
                rows) and AllGather it for layer l (hfull index l-1)."""
                nc.sync.dma_start(
                    in_cc_q[q][:].rearrange("p (b e) -> p b e", e=EL8)
                        [:, :, 0:H],
                    ht8_sb[:][:, int(b0q[q]):int(b0q[q + 1]), :])
                if "ag" not in skip:
                    nc.gpsimd.collective_compute(
                        "AllGather", AL.bypass, replica_groups=RG,
                        ins=[in_cc_q[q].opt()],
                        outs=[hfull_rlq[r][l - 1][q].opt()])

            qend = {int(b0q[q + 1]) - 1: q for q in range(NWIN)}

            for _rep in range(repeat):
              hfull = hfull_rlq[_rep]
              prd_out = prd_out_r[_rep]

              # layer-1 input: h~ = dinv * x (bf16) -- self-loop source only;
              # layer 0's gather tables are host inputs (no AllGather)
              for bi in range(nblk):
                if "hscale" not in skip:
                    nc.vector.tensor_scalar(
                        out=ht_sb[:][:, bi, 0:H],
                        in0=xp_sb[:][:, bi * H:(bi + 1) * H],
                        scalar1=dinv_sb[:][:, bi:bi + 1], scalar2=None,
                        op0=AL.mult)

              poolT = fin_ps.tile([H, P], f32, tag="poolT")
              for l in range(3):
                last = l == 2
                if l == 0:
                    gat = [x0t_d[q].ap()
                               .rearrange("p (b e) -> (p b) e", e=EL8)
                           for q in range(NWIN)]
                else:
                    gat = [hfull[l - 1][q][:]
                               .rearrange("p (b e) -> (p b) e", e=EL8)
                           for q in range(NWIN)]

                call_no = 0
                for sg in range(nsg):
                    tok0 = sg_tok0[sg]
                    sg_ntok = sg_w_ranges[sg][-1][2] - tok0
                    msgs = msgs_tp.tile([P, sg_ntok // P, H], f8,
                                        tag="msgs", name="msgs")
                    for (ww, t0, t1) in sg_w_ranges[sg]:
                        t = t0
                        while t < t1:
                            tc_ = min(tmax, t1 - t)
                            if "gather" in skip:
                                t += tc_
                                continue
                            dma_gather_any(
                                nc.gpsimd,
                                out_ap=msgs[:][:, (t - tok0) // P:
                                               (t - tok0 + tc_) // P, :],
                                in_ap=gat[ww][:, 0:H],
                                idxs_ap=eidx_sb[:][:, t // 16:(t + tc_) // 16],
                                num_idxs=tc_, num_idxs_reg=tc_,
                                elem_size=H, elem_step=EL8,
                                single_packet=single_packet,
                                queue_num=call_no % nq)
                            call_no += 1
                            t += tc_
                    for bi in range(sg * sgblk, (sg + 1) * sgblk):
                        aggT = agg_ps.tile([H, P], f32, tag="agg", name="agg")
                        kbt = sum(spans[(bi, w)][1] - spans[(bi, w)][0]
                                  for w in range(NWIN))
                        nmm = 1 + kbt
                        if "mm" not in skip:
                            # self-loop: aggT[f, d] += ht[d, b, f]
                            nc.tensor.matmul(
                                aggT[:], lhsT=ht_sb[:][:, bi, 0:H],
                                rhs=ident_bf[:], start=True, stop=False)
                        imm = 1
                        cw0 = col2_of[(bi, 0)]
                        MB = m_tp.tile([P, kbt * P], f8, tag="MB",
                                       name="MB")
                        if "mb" not in skip:
                            nc.vector.tensor_tensor(
                                out=MB[:].rearrange("p (c q) -> p c q", q=P),
                                in0=edst_sb[:][:, cw0:cw0 + kbt]
                                    .to_broadcast([P, kbt, P]),
                                in1=iota_bf[:][:, None, :]
                                    .to_broadcast([P, kbt, P]),
                                op=AL.is_equal)
                        MBr = MB[:].rearrange("p (c q) -> p c q", q=P)
                        for w in range(NWIN):
                            c0, c1 = spans[(bi, w)]
                            kb = c1 - c0
                            joff = col2_of[(bi, w)] - cw0
                            j = 0
                            while j < kb:
                                mc = c0 + j - tok0 // P
                                pair = j + 1 < kb
                                nj = 2 if pair else 1
                                if "mm" in skip:
                                    imm += nj
                                    j += nj
                                    continue
                                if pair:
                                    nc.tensor.matmul(
                                        aggT[:],
                                        lhsT=msgs[:][:, mc:mc + 2, :],
                                        rhs=MBr[:, joff + j:joff + j + 2, :],
                                        start=(imm == 0),
                                        stop=(imm + 2 == nmm),
                                        perf_mode=(
                                            mybir.MatmulPerfMode.DoubleRow),
                                    )
                                else:
                                    nc.tensor.matmul(
                                        aggT[:],
                                        lhsT=msgs[:][:, mc, :],
                                        rhs=MBr[:, joff + j, :],
                                        start=(imm == 0),
                                        stop=(imm + 1 == nmm))
                                imm += nj
                                j += nj
                        if "epi" in skip:
                            continue
                        sT = s_tp.tile([H, P], f32, tag="sT", name="sT")
                        nc.scalar.copy(out=sT[:], in_=aggT[:])
                        outb = out_ps.tile([P, H], f32, tag="outb",
                                           name="outb")
                        nc.tensor.matmul(outb[:], lhsT=sT[:], rhs=w_sb[l][:],
                                         start=True, stop=True)
                        dcol = dinv_sb[:][:, bi:bi + 1]
                        t1_ = e_tp.tile([P, H], f32, tag="t1", name="t1")
                        nc.vector.tensor_scalar(
                            out=t1_[:], in0=outb[:], scalar1=dcol,
                            scalar2=None, op0=AL.mult)
                        if not last:
                            t2 = e_tp.tile([P, H], f32, tag="t2", name="t2")
                            nc.vector.tensor_tensor(
                                out=t2[:], in0=t1_[:], in1=bias_sb[l][:],
                                op=AL.add)
                            nc.vector.tensor_scalar(
                                out=ht_sb[:][:, bi, 0:H], in0=t2[:],
                                scalar1=0.0, scalar2=dcol,
                                op0=AL.max, op1=AL.mult)
                            nc.vector.tensor_scalar(
                                out=ht8_sb[:][:, bi, :], in0=t2[:],
                                scalar1=0.0, scalar2=dcol,
                                op0=AL.max, op1=AL.mult)
                            if bi in qend:
                                issue_ag(_rep, l + 1, qend[bi])
                        else:
                            nc.vector.tensor_tensor(
                                out=h3_sb[:][:, bi * H:(bi + 1) * H],
                                in0=t1_[:], in1=bias_sb[l][:], op=AL.add)

              # pooling: poolT[f, g] = sum_n h3[n, f] * (batch[n] == g)
              for bi in range(nblk):
                Mg = m_tp.tile([P, P], f32, tag="Mg", name="Mg")
                nc.vector.tensor_scalar(
                    out=Mg[:], in0=iota_f[:],
                    scalar1=batg_sb[:][:, bi:bi + 1], scalar2=None,
                    op0=AL.is_equal)
                nc.tensor.matmul(poolT[:],
                                 lhsT=h3_sb[:][:, bi * H:(bi + 1) * H],
                                 rhs=Mg[:], start=(bi == 0),
                                 stop=(bi == nblk - 1))
              poolT_sb = s_tp.tile([H, P], f32, tag="poolTs")
              nc.vector.tensor_copy(out=poolT_sb[:], in_=poolT[:])
              nc.sync.dma_start(prd_in[:], poolT_sb[:])
              nc.gpsimd.collective_compute(
                  "AllReduce", AL.add, replica_groups=RG,
                  ins=[prd_in.opt()], outs=[prd_out.opt()])
              poolF = s_tp.tile([H, P], f32, tag="poolF")
              nc.sync.dma_start(poolF[:], prd_out[:])
              fin = fin_ps.tile([P, C], f32, tag="fin")
              nc.tensor.matmul(fin[:], lhsT=poolF[:], rhs=wl_sb[:],
                               start=True, stop=True)
              outf = e_tp.tile([P, C], f32, tag="outf")
              nc.vector.tensor_scalar(out=outf[:], in0=fin[:],
                                      scalar1=cinv_sb[:], scalar2=None,
                                      op0=AL.mult)
              outf2 = e_tp.tile([P, C], f32, tag="outf2")
              nc.vector.tensor_tensor(out=outf2[:], in0=outf[:],
                                      in1=biasl_sb[:], op=AL.add)
              nc.sync.dma_start(out_d.ap()[:, :], outf2[:][:G, :])

    nc.compile()
    return nc


def make_in_maps(pp, weights, G, n_cores=NCORES):
    W1, b1, W2, b2, W3, b3, Wl, bl, counts = weights
    H = pp["H"]
    C = np.asarray(Wl).shape[1]
    bf = ml_dtypes.bfloat16
    iota_row = np.arange(P, dtype=np.float32)
    iota_bf = np.ascontiguousarray(np.broadcast_to(iota_row, (P, P))).astype(bf)
    iota_f = np.ascontiguousarray(np.broadcast_to(iota_row, (P, P)))
    cinv = np.ones((P, 1), np.float32)
    cinv[:G, 0] = 1.0 / np.maximum(counts, 1.0)
    shared = {
        "iota_bf": iota_bf, "iota_f": iota_f,
        "ident": np.eye(P, dtype=np.float32).astype(bf),
        **{f"x0t{q}": pp["x0t"][q] for q in range(NWIN)},
        "w0": np.asarray(W1, np.float32), "w1": np.asarray(W2, np.float32),
        "w2": np.asarray(W3, np.float32),
        "bias0": np.ascontiguousarray(np.broadcast_to(b1, (P, H))).astype(np.float32),
        "bias1": np.ascontiguousarray(np.broadcast_to(b2, (P, H))).astype(np.float32),
        "bias2": np.ascontiguousarray(np.broadcast_to(b3, (P, H))).astype(np.float32),
        "wl": np.asarray(Wl, np.float32),
        "biasl": np.ascontiguousarray(np.broadcast_to(bl, (P, C))).astype(np.float32),
        "cinv": cinv,
    }
    maps = []
    for c in range(n_cores):
        m = dict(shared)
        m["eidx"] = pp["eidx16"][c]
        m["edst"] = pp["edst2"][c].astype(bf)
        m["xp"] = pp["xp"][c]
        m["dinv"] = pp["dinv"][c]
        m["batg"] = pp["batg"][c]
        maps.append(m)
    return maps


LAST_RESULT = None
LAST_NC = None
LAST_IN_MAPS = None
LAST_BUILD = None


def kernel(x, edge_index, batch, W1, b1, W2, b2, W3, b3, Wl, bl, **run_kwargs):
    """Full-input entry point. Shards across 8 cores, runs on HW, gathers."""
    global LAST_RESULT, LAST_NC, LAST_IN_MAPS, LAST_BUILD
    from concourse.bass_utils import run_bass_kernel_spmd

    x = np.asarray(x, np.float32)
    edge_index = np.asarray(edge_index)
    batch = np.asarray(batch)
    G = G_FULL
    C = np.asarray(Wl).shape[1]

    pp = preprocess(x, edge_index, batch)
    counts = np.bincount(batch.astype(np.int64), minlength=G).astype(np.float32)
    nc = build_nc(pp, G, C)
    in_maps = make_in_maps(pp, (W1, b1, W2, b2, W3, b3, Wl, bl, counts), G)
    res = run_bass_kernel_spmd(nc, in_maps, core_ids=list(range(NCORES)),
                               **run_kwargs)
    LAST_RESULT, LAST_NC, LAST_IN_MAPS = res, nc, in_maps
    LAST_BUILD = dict(pp=pp, G=G, C=C)
    return res.results[0]["out"].astype(np.float32)


# revision 31
# speedup vs baseline: 1.0500x; 1.0017x over previous
"""3-layer GCN + global mean pool + linear head on 8 Trainium2 NeuronCores.

Strategy (dst-sharded message passing, v2):
  - GCN normalization factorizes: norm_e = dinv[src]*dinv[dst], so each conv
    layer is  h' = relu( dinv * ((Adj+I) @ (dinv * h)) @ W + b ).  Only pure
    row gather + segment-sum on device; diagonal scalings are per-node ops.
  - Nodes (and their in-edges, self-loops appended) are sharded across the 8
    cores by contiguous dst ranges.
  - The h~ table is quartered BY BLOCK RANGE (window w = block quarter q of
    every core's slice).  Each quarter is AllGathered separately as soon as
    the previous layer finishes that quarter's blocks, so collectives hide
    under compute.  Gather windows == quarters; int16 indices are relative
    to the window table.
  - Edge stream per core: for sg (7 dst blocks): for w (rotated by sg so the
    first gathers of a layer spread across all four AllGathers): edges of
    the sg's blocks in window w, concatenated unpadded, group padded to 128
    tokens UNIFORMLY across cores (max-core length).  The h~ tables are
    fp8e4 (rows: 64B payload + 192B pad at the mandatory 256B stride);
    dma_gather fetches just the 64B payloads (elem 64 x fp8, stride 256B),
    <=1024 tokens per call, rotated over 4 SWDGE queues.
  - Segment-sum on the TensorEngine in fp8 DoubleRow mode: chunk PAIRS per
    instruction, aggT[64f,128d] += [2x128e,64f].T @ M[2x128e,128d], M built
    by a broadcast is_equal of edst2 (per-(block,window,chunk) dst_rel or
    -1, fp8 out) against an iota tile.  Chunk spans are uniform across cores (min/max
    over cores); out-of-block tokens carry -1 and contribute zero.
  - The layer weight applies after aggregation, then dinv/bias/relu DVE ops.
  - Mean-pool uses the same one-hot matmul against graph ids; partials are
    AllReduced; the head matmul finishes on every core.
"""
import numpy as np
import ml_dtypes

P = 128
NCORES = 8
NWIN = 4          # block-quarter windows
TMAX = 1024       # max tokens per dma_gather call (SWDGE ring limit)
SGBLK = 7         # dst blocks per super-group (msgs buffer granularity)

# Full-size problem dims (nn_GCN_13881334300836)
N_FULL, E_FULL, D_FULL, C_FULL, G_FULL = 100_000, 1_250_000, 64, 10, 128


# --------------------------------------------------------------------------
# Host preprocessing
# --------------------------------------------------------------------------

def preprocess(x, edge_index, batch, n_cores=NCORES):
    """Shard nodes/edges; build window-rotated, group-padded gather streams
    with cross-core-uniform chunk spans.

    Table row for node n (c=n//npc, local=n-c*npc, p=local%P, b=local//P,
    q=quarter(b)): window q, row (c*P + p)*nbq[q] + (b - b0[q]).
    """
    N, H = x.shape
    assert N % n_cores == 0
    npc = N // n_cores
    nblk = (npc + P - 1) // P
    npad = nblk * P

    # block quarters (windows); the LAST quarter is small so the one
    # AllGather that cannot hide (issued after the previous layer's final
    # block) is cheap
    last_q = max(2, nblk // 12)
    base, rem = divmod(nblk - last_q, NWIN - 1)
    nbq = [base + (1 if q < rem else 0) for q in range(NWIN - 1)] + [last_q]
    b0q = np.cumsum([0] + nbq)          # len NWIN+1
    wrows = [n_cores * P * nbq[q] for q in range(NWIN)]
    assert max(wrows) <= 32768

    sgblk = SGBLK
    if nblk % sgblk != 0:
        sgblk = next((g for g in (7, 8, 6, 5, 4, 9, 10, 3, 2) if nblk % g == 0),
                     nblk)
    nsg = nblk // sgblk

    ei = edge_index.astype(np.int64)
    # degrees include self-loops (PyG gcn_norm), but self-loop messages are
    # local (identity matmul on-device) -- exclude them from the stream
    src_all = ei[0]
    dst_all = ei[1]

    deg = (np.bincount(dst_all, minlength=N) + 1).astype(np.float32)
    dinv = (1.0 / np.sqrt(np.maximum(deg, 1.0))).astype(np.float32)

    # source -> (window, idx16)
    core_of = src_all // npc
    local = src_all - core_of * npc
    p_of = local % P
    b_of = local // P
    q_of = np.searchsorted(b0q, b_of, side="right") - 1
    nbq_arr = np.asarray(nbq)
    idx_of = (core_of * P + p_of) * nbq_arr[q_of] + (b_of - b0q[q_of])

    # per-core sorted edge lists: key (dst block, window), stable
    core_edges = []   # c -> (off, s16, dr)
    cnt_all = np.zeros((n_cores, nblk, NWIN), np.int64)
    for c in range(n_cores):
        lo = c * npc
        m = (dst_all >= lo) & (dst_all < lo + npc)
        s16 = idx_of[m].astype(np.int16)
        w = q_of[m]
        d = dst_all[m] - lo
        db, dr = d // P, (d % P).astype(np.float32)
        key = db * NWIN + w
        order = np.argsort(key, kind="stable")
        s16, dr, key = s16[order], dr[order], key[order]
        cnt = np.bincount(key, minlength=nblk * NWIN).reshape(nblk, NWIN)
        cnt_all[c] = cnt
        off = np.zeros(nblk * NWIN + 1, np.int64)
        np.cumsum(cnt.ravel(), out=off[1:])
        core_edges.append((off, s16, dr))

    # uniform group lengths: for (sg, w): tokens = max_c sum_b cnt, pad to 128
    grp_len = np.zeros((nsg, NWIN), np.int64)
    for sg in range(nsg):
        bs = list(range(sg * sgblk, (sg + 1) * sgblk))
        for w in range(NWIN):
            mx = max(int(cnt_all[c, bs, w].sum()) for c in range(n_cores))
            grp_len[sg, w] = -(-max(mx, 1) // P) * P

    # stream layout: for sg: for w in rot(sg): group
    rot = [[(sg + j) % NWIN for j in range(NWIN)] for sg in range(nsg)]
    grp_tok0 = np.zeros((nsg, NWIN), np.int64)   # token start of (sg, w)
    sg_tok0 = []
    sg_w_ranges = []   # sg -> [(w, tok0, tok1)] in rotated order
    pos = 0
    for sg in range(nsg):
        sg_tok0.append(pos)
        rngs = []
        for w in rot[sg]:
            grp_tok0[sg, w] = pos
            rngs.append((w, pos, pos + int(grp_len[sg, w])))
            pos += int(grp_len[sg, w])
        sg_w_ranges.append(rngs)
    ntok = pos
    nchunk = ntok // P

    # per-(b, w) uniform chunk spans + matmul metadata
    spans = {}          # (b, w) -> (c0, c1)  global chunk ids
    ncol2 = 0
    col2_of = {}        # (b, w) -> starting edst2 column
    for sg in range(nsg):
        for b in range(sg * sgblk, (sg + 1) * sgblk):
            for w in range(NWIN):
                g0 = int(grp_tok0[sg, w])
                t0s, t1s = [], []
                for c in range(n_cores):
                    pre = int(cnt_all[c, sg * sgblk:b, w].sum())
                    cn = int(cnt_all[c, b, w])
                    t0s.append(g0 + pre)
                    t1s.append(g0 + pre + cn)
                c0 = min(t0s) // P
                c1 = -(-max(t1s) // P)
                c1 = max(c1, c0 + 1)
                spans[(b, w)] = (int(c0), int(c1))
                col2_of[(b, w)] = ncol2
                ncol2 += int(c1 - c0)

    # build per-core streams + edst2
    eidx16 = np.zeros((n_cores, 16, ntok // 16), np.int16)
    edst2 = np.full((n_cores, P, ncol2), -1.0, np.float32)
    for c in range(n_cores):
        off, s16, dr = core_edges[c]
        stream = np.zeros(ntok, np.int16)
        drel = np.full(ntok, -1.0, np.float32)   # dst_rel per token
        bof = np.full(ntok, -1, np.int64)        # owning block per token
        for sg in range(nsg):
            for w in range(NWIN):
                t = int(grp_tok0[sg, w])
                for b in range(sg * sgblk, (sg + 1) * sgblk):
                    k = b * NWIN + w
                    sl = slice(off[k], off[k + 1])
                    n_e = int(off[k + 1] - off[k])
                    stream[t:t + n_e] = s16[sl]
                    drel[t:t + n_e] = dr[sl]
                    bof[t:t + n_e] = b
                    t += n_e
        eidx16[c] = stream.reshape(ntok // 16, 16).T
        for (b, w), (c0, c1) in spans.items():
            cw = col2_of[(b, w)]
            seg_d = drel[c0 * P:c1 * P].copy()
            seg_b = bof[c0 * P:c1 * P]
            seg_d[seg_b != b] = -1.0
            edst2[c][:, cw:cw + (c1 - c0)] = seg_d.reshape(c1 - c0, P).T

    dinv_pc = np.zeros((n_cores, P, nblk), np.float32)
    bat_pc = np.full((n_cores, P, nblk), -1.0, np.float32)
    xp_pc = np.zeros((n_cores, P, nblk * H), np.float32)
    xf = np.asarray(x, np.float32)
    for c in range(n_cores):
        dv = np.zeros(npad, np.float32)
        dv[:npc] = dinv[c * npc:(c + 1) * npc]
        dinv_pc[c] = dv.reshape(nblk, P).T
        bt = np.full(npad, -1.0, np.float32)
        bt[:npc] = batch[c * npc:(c + 1) * npc].astype(np.float32)
        bat_pc[c] = bt.reshape(nblk, P).T
        xp = np.zeros((npad, H), np.float32)
        xp[:npc] = xf[c * npc:(c + 1) * npc]
        xp_pc[c] = xp.reshape(nblk, P, H).transpose(1, 0, 2).reshape(P, nblk * H)

    # layer-0 gather tables: h~0 = dinv * x for ALL nodes, window-laid-out
    # (host-computable, so layer 0 needs no AllGather at runtime).
    # fp8e4 rows: [64 B payload | 192 B pad] at the required 256 B stride.
    import concourse.mybir as _mybir
    f8d = _mybir.dt.np(_mybir.dt.float8e4)
    ht0 = (xf * dinv[:, None]).astype(f8d)
    n_all = np.arange(N)
    cn = n_all // npc
    ln = n_all - cn * npc
    pn, bn = ln % P, ln // P
    qn = np.searchsorted(b0q, bn, side="right") - 1
    rn = (cn * P + pn) * nbq_arr[qn] + (bn - b0q[qn])
    x0t = []
    for q in range(NWIN):
        Tq = np.zeros((n_cores * P * nbq[q], 2 * P), f8d)
        mq = qn == q
        Tq[rn[mq], :H] = ht0[n_all[mq]]
        x0t.append(Tq.reshape(n_cores * P, nbq[q] * 2 * P))

    return dict(eidx16=eidx16, edst2=edst2, dinv=dinv_pc, batg=bat_pc,
                xp=xp_pc, x0t=x0t, npc=npc, nblk=nblk, nsg=nsg, sgblk=sgblk,
                ntok=ntok, nchunk=nchunk, ncol2=ncol2, spans=spans,
                col2_of=col2_of, sg_tok0=sg_tok0, sg_w_ranges=sg_w_ranges,
                grp_tok0=grp_tok0, nbq=nbq, b0q=b0q, wrows=wrows, H=H)


# --------------------------------------------------------------------------
# Device kernel builder
# --------------------------------------------------------------------------

def dma_gather_any(gp, out_ap, in_ap, idxs_ap, num_idxs, num_idxs_reg,
                   elem_size, elem_step, single_packet=True, queue_num=0):
    """dma_gather with elem_size_bytes not restricted to %256 (non-transpose
    HBM-source path only; stride (elem_step) must still be a 256B multiple).
    Mirrors bass.GpSimd.dma_gather minus the transpose-only elem assert."""
    import concourse.mybir as mybir
    from concourse import ap_utils
    from concourse.bass import exact_div

    gp._assert_queue_num(queue_num)
    assert idxs_ap.dtype == mybir.dt.int16
    assert in_ap.dtype == out_ap.dtype
    elem_size_bytes = elem_size * mybir.dt.size(in_ap.dtype)
    assert elem_size_bytes > 0
    assert in_ap.space.name == "DRAM"
    assert idxs_ap.space.name == "SBUF"
    assert out_ap.space.name == "SBUF"
    assert ap_utils.ap_is_contiguous(out_ap.ap[1:])
    assert ap_utils.ap_is_contiguous(idxs_ap.ap[1:])
    assert in_ap.ap[-1][1] == out_ap.ap[-1][1] == elem_size
    assert out_ap.ap[0][1] * out_ap.ap[1][1] % 128 == 0
    assert in_ap.ap[0][0] == elem_step
    stride_bytes = elem_step * mybir.dt.size(in_ap.dtype)
    stride_bytes_256 = exact_div(stride_bytes, 256)
    assert stride_bytes_256 < 256
    _in_ap = gp.lower_ap_dma(in_ap, for_custom_bir_dma=True)
    _idxs_ap = gp.lower_ap(idxs_ap)
    _out_ap = gp.lower_ap(out_ap)
    return gp.add_instruction(
        mybir.InstDMAGatherAnt(
            name=gp.bass.get_next_instruction_name(),
            ins=[*_in_ap, _idxs_ap,
                 gp.lower_val_access(gp.to_reg(num_idxs_reg))],
            outs=[_out_ap],
            transpose=False,
            num_idxs=num_idxs,
            elem_size=elem_size,
            stride_bytes_256=stride_bytes_256,
            gen_mode=0,
            single_packet=single_packet,
            queue_num=queue_num,
            sbuf_tokens_per_rank=0,
            sbuf_free_dim_per_rank=0,
            sbuf_free_dim_pad_per_rank=0,
            sbuf_byte_offset=0,
        ))


def build_nc(pp, G, C, n_cores=NCORES, repeat=1, skip=frozenset(),
             nq=4, tmax=TMAX, single_packet=True):
    """Build the Bass program (shared SPMD across n_cores).

    repeat>1 re-runs the whole forward pass that many times inside one NEFF
    (delta-method HW timing only).  skip: timing-experiment knob."""
    import concourse.bacc as bacc
    import concourse.mybir as mybir
    import concourse.tile as tile
    from contextlib import ExitStack

    H = pp["H"]
    nblk, nsg, sgblk = pp["nblk"], pp["nsg"], pp["sgblk"]
    ntok, nchunk, ncol2 = pp["ntok"], pp["nchunk"], pp["ncol2"]
    spans, col2_of = pp["spans"], pp["col2_of"]
    sg_tok0, sg_w_ranges = pp["sg_tok0"], pp["sg_w_ranges"]
    nbq, b0q = pp["nbq"], pp["b0q"]
    RG = [list(range(n_cores))]
    EL = P  # padded table row width in bf16 elements (256B rows)

    f32, bf16 = mybir.dt.float32, mybir.dt.bfloat16
    i16 = mybir.dt.int16
    AL = mybir.AluOpType

    nc = bacc.Bacc("TRN2", target_bir_lowering=False, debug=False,
                   enable_asserts=False, num_devices=n_cores,
                   num_swdge_queues=nq)

    eidx_d = nc.dram_tensor("eidx", [16, ntok // 16], i16, kind="ExternalInput")
    edst_d = nc.dram_tensor("edst", [P, ncol2], bf16, kind="ExternalInput")
    xp_d = nc.dram_tensor("xp", [P, nblk * H], f32, kind="ExternalInput")
    dinv_d = nc.dram_tensor("dinv", [P, nblk], f32, kind="ExternalInput")
    batg_d = nc.dram_tensor("batg", [P, nblk], f32, kind="ExternalInput")
    iota_bf_d = nc.dram_tensor("iota_bf", [P, P], bf16, kind="ExternalInput")
    ident_d = nc.dram_tensor("ident", [P, P], bf16, kind="ExternalInput")
    iota_f_d = nc.dram_tensor("iota_f", [P, P], f32, kind="ExternalInput")
    w_d = [nc.dram_tensor(f"w{l}", [H, H], f32, kind="ExternalInput")
           for l in range(3)]
    bias_d = [nc.dram_tensor(f"bias{l}", [P, H], f32, kind="ExternalInput")
              for l in range(3)]
    wl_d = nc.dram_tensor("wl", [H, C], f32, kind="ExternalInput")
    biasl_d = nc.dram_tensor("biasl", [P, C], f32, kind="ExternalInput")
    cinv_d = nc.dram_tensor("cinv", [P, 1], f32, kind="ExternalInput")
    f8 = mybir.dt.float8e4
    EL8 = 2 * P   # fp8 elems per 256B table row
    x0t_d = [nc.dram_tensor(f"x0t{q}", [n_cores * P, nbq[q] * EL8], f8,
                            kind="ExternalInput") for q in range(NWIN)]
    out_d = nc.dram_tensor("out", [G, C], f32, kind="ExternalOutput")

    with tile.TileContext(nc) as tc:
        with ExitStack() as ctx:
            const = ctx.enter_context(tc.tile_pool(name="const", bufs=1))
            msgs_tp = ctx.enter_context(tc.tile_pool(name="msgs", bufs=2))
            m_tp = ctx.enter_context(tc.tile_pool(name="mb", bufs=3))
            s_tp = ctx.enter_context(tc.tile_pool(name="st", bufs=3))
            e_tp = ctx.enter_context(tc.tile_pool(name="ep", bufs=4))
            agg_ps = ctx.enter_context(tc.tile_pool(name="aggp", bufs=4,
                                                    space="PSUM"))
            out_ps = ctx.enter_context(tc.tile_pool(name="outp", bufs=2,
                                                    space="PSUM"))
            fin_ps = ctx.enter_context(tc.tile_pool(name="finp", bufs=1,
                                                    space="PSUM"))
            dram = ctx.enter_context(tc.tile_pool(name="dram", bufs=1,
                                                  space="DRAM"))

            eidx_sb = const.tile([128, ntok // 16], i16)
            edst_sb = const.tile([P, ncol2], bf16)
            iota_bf = const.tile([P, P], bf16)
            iota_f = const.tile([P, P], f32)
            ident_bf = const.tile([P, P], bf16)
            dinv_sb = const.tile([P, nblk], f32)
            batg_sb = const.tile([P, nblk], f32)
            w_sb = [const.tile([H, H], f32, tag=f"w{l}", name=f"w{l}_sb")
                    for l in range(3)]
            bias_sb = [const.tile([P, H], f32, tag=f"b{l}", name=f"b{l}_sb")
                       for l in range(3)]
            wl_sb = const.tile([H, C], f32)
            biasl_sb = const.tile([P, C], f32)
            cinv_sb = const.tile([P, 1], f32)
            ht_sb = const.tile([P, nblk, EL], bf16)   # h~ slice, 256B rows
            ht8_sb = const.tile([P, nblk, H], f8)     # fp8 h~ (64B payload)
            h3_sb = const.tile([P, nblk * H], f32)
            xp_sb = const.tile([P, nblk * H], f32)

            # idx tile: replicate the [16, S] wrap to all 8 partition groups
            for g8 in range(8):
                nc.sync.dma_start(eidx_sb[:][g8 * 16:(g8 + 1) * 16, :],
                                  eidx_d.ap())
            nc.sync.dma_start(edst_sb[:], edst_d.ap())
            nc.sync.dma_start(iota_bf[:], iota_bf_d.ap())
            nc.sync.dma_start(ident_bf[:], ident_d.ap())
            nc.sync.dma_start(iota_f[:], iota_f_d.ap())
            nc.sync.dma_start(dinv_sb[:], dinv_d.ap())
            nc.sync.dma_start(batg_sb[:], batg_d.ap())
            for l in range(3):
                nc.sync.dma_start(w_sb[l][:], w_d[l].ap())
                nc.sync.dma_start(bias_sb[l][:], bias_d[l].ap())
            nc.sync.dma_start(wl_sb[:], wl_d.ap())
            nc.sync.dma_start(biasl_sb[:], biasl_d.ap())
            nc.sync.dma_start(cinv_sb[:], cinv_d.ap())
            nc.sync.dma_start(xp_sb[:], xp_d.ap())
            # zero the padding feature columns of h~ once
            nc.vector.memset(ht_sb[:], 0.0)
            if "epi" in skip:
                nc.vector.memset(h3_sb[:], 0.0)

            # per-quarter staging + per-(repeat, layer, quarter) shared outs
            in_cc_q = [dram.tile([P, nbq[q] * EL8], f8, tag=f"incc{q}",
                                 name=f"incc{q}") for q in range(NWIN)]
            hfull_rlq = [[[dram.tile([n_cores * P, nbq[q] * EL8], f8,
                                     addr_space="Shared",
                                     tag=f"hf{r}_{l}_{q}",
                                     name=f"hf{r}_{l}_{q}")
                           for q in range(NWIN)] for l in (1, 2)]
                         for r in range(repeat)]
            prd_in = dram.tile([H, P], f32)
            prd_out_r = [dram.tile([H, P], f32, addr_space="Shared",
                                   tag=f"prd_out_{r}", name=f"prd_out_{r}")
                         for r in range(repeat)]

            def issue_ag(r, l, q):
                """Stage quarter q of ht8 (fp8 payload halves of the 256B
                rows) and AllGather it for layer l (hfull index l-1)."""
                nc.sync.dma_start(
                    in_cc_q[q][:].rearrange("p (b e) -> p b e", e=EL8)
                        [:, :, 0:H],
                    ht8_sb[:][:, int(b0q[q]):int(b0q[q + 1]), :])
                if "ag" not in skip:
                    nc.gpsimd.collective_compute(
                        "AllGather", AL.bypass, replica_groups=RG,
                        ins=[in_cc_q[q].opt()],
                        outs=[hfull_rlq[r][l - 1][q].opt()])

            qend = {int(b0q[q + 1]) - 1: q for q in range(NWIN)}

            for _rep in range(repeat):
              hfull = hfull_rlq[_rep]
              prd_out = prd_out_r[_rep]

              # layer-1 input: h~ = dinv * x (bf16) -- self-loop source only;
              # layer 0's gather tables are host inputs (no AllGather)
              for bi in range(nblk):
                if "hscale" not in skip:
                    nc.vector.tensor_scalar(
                        out=ht_sb[:][:, bi, 0:H],
                        in0=xp_sb[:][:, bi * H:(bi + 1) * H],
                        scalar1=dinv_sb[:][:, bi:bi + 1], scalar2=None,
                        op0=AL.mult)

              poolT = fin_ps.tile([H, P], f32, tag="poolT")
              for l in range(3):
                last = l == 2
                if l == 0:
                    gat = [x0t_d[q].ap()
                               .rearrange("p (b e) -> (p b) e", e=EL8)
                           for q in range(NWIN)]
                else:
                    gat = [hfull[l - 1][q][:]
                               .rearrange("p (b e) -> (p b) e", e=EL8)
                           for q in range(NWIN)]

                call_no = 0
                for sg in range(nsg):
                    tok0 = sg_tok0[sg]
                    sg_ntok = sg_w_ranges[sg][-1][2] - tok0
                    msgs = msgs_tp.tile([P, sg_ntok // P, H], f8,
                                        tag="msgs", name="msgs")
                    for (ww, t0, t1) in sg_w_ranges[sg]:
                        t = t0
                        while t < t1:
                            tc_ = min(tmax, t1 - t)
                            if "gather" in skip:
                                t += tc_
                                continue
                            dma_gather_any(
                                nc.gpsimd,
                                out_ap=msgs[:][:, (t - tok0) // P:
                                               (t - tok0 + tc_) // P, :],
                                in_ap=gat[ww][:, 0:H],
                                idxs_ap=eidx_sb[:][:, t // 16:(t + tc_) // 16],
                                num_idxs=tc_, num_idxs_reg=tc_,
                                elem_size=H, elem_step=EL8,
                                single_packet=single_packet,
                                queue_num=call_no % nq)
                            call_no += 1
                            t += tc_
                    for bi in range(sg * sgblk, (sg + 1) * sgblk):
                        aggT = agg_ps.tile([H, P], f32, tag="agg", name="agg")
                        kbt = sum(spans[(bi, w)][1] - spans[(bi, w)][0]
                                  for w in range(NWIN))
                        nmm = 1 + kbt
                        if "mm" not in skip:
                            # self-loop: aggT[f, d] += ht[d, b, f]
                            nc.tensor.matmul(
                                aggT[:], lhsT=ht_sb[:][:, bi, 0:H],
                                rhs=ident_bf[:], start=True, stop=False)
                        imm = 1
                        cw0 = col2_of[(bi, 0)]
                        MB = m_tp.tile([P, kbt * P], f8, tag="MB",
                                       name="MB")
                        if "mb" not in skip:
                            nc.vector.tensor_tensor(
                                out=MB[:].rearrange("p (c q) -> p c q", q=P),
                                in0=edst_sb[:][:, cw0:cw0 + kbt]
                                    .to_broadcast([P, kbt, P]),
                                in1=iota_bf[:][:, None, :]
                                    .to_broadcast([P, kbt, P]),
                                op=AL.is_equal)
                        MBr = MB[:].rearrange("p (c q) -> p c q", q=P)
                        for w in range(NWIN):
                            c0, c1 = spans[(bi, w)]
                            kb = c1 - c0
                            joff = col2_of[(bi, w)] - cw0
                            j = 0
                            while j < kb:
                                mc = c0 + j - tok0 // P
                                pair = j + 1 < kb
                                nj = 2 if pair else 1
                                if "mm" in skip:
                                    imm += nj
                                    j += nj
                                    continue
                                if pair:
                                    nc.tensor.matmul(
                                        aggT[:],
                                        lhsT=msgs[:][:, mc:mc + 2, :],
                                        rhs=MBr[:, joff + j:joff + j + 2, :],
                                        start=(imm == 0),
                                        stop=(imm + 2 == nmm),
                                        perf_mode=(
                                            mybir.MatmulPerfMode.DoubleRow),
                                    )
                                else:
                                    nc.tensor.matmul(
                                        aggT[:],
                                        lhsT=msgs[:][:, mc, :],
                                        rhs=MBr[:, joff + j, :],
                                        start=(imm == 0),
                                        stop=(imm + 1 == nmm))
                                imm += nj
                                j += nj
                        if "epi" in skip:
                            continue
                        sT = s_tp.tile([H, P], f32, tag="sT", name="sT")
                        nc.scalar.copy(out=sT[:], in_=aggT[:])
                        outb = out_ps.tile([P, H], f32, tag="outb",
                                           name="outb")
                        nc.tensor.matmul(outb[:], lhsT=sT[:], rhs=w_sb[l][:],
                                         start=True, stop=True)
                        dcol = dinv_sb[:][:, bi:bi + 1]
                        t1_ = e_tp.tile([P, H], f32, tag="t1", name="t1")
                        nc.vector.tensor_scalar(
                            out=t1_[:], in0=outb[:], scalar1=dcol,
                            scalar2=None, op0=AL.mult)
                        if not last:
                            t2 = e_tp.tile([P, H], f32, tag="t2", name="t2")
                            nc.vector.tensor_tensor(
                                out=t2[:], in0=t1_[:], in1=bias_sb[l][:],
                                op=AL.add)
                            nc.vector.tensor_scalar(
                                out=ht_sb[:][:, bi, 0:H], in0=t2[:],
                                scalar1=0.0, scalar2=dcol,
                                op0=AL.max, op1=AL.mult)
                            nc.vector.tensor_scalar(
                                out=ht8_sb[:][:, bi, :], in0=t2[:],
                                scalar1=0.0, scalar2=dcol,
                                op0=AL.max, op1=AL.mult)
                            if bi in qend:
                                issue_ag(_rep, l + 1, qend[bi])
                        else:
                            nc.vector.tensor_tensor(
                                out=h3_sb[:][:, bi * H:(bi + 1) * H],
                                in0=t1_[:], in1=bias_sb[l][:], op=AL.add)

              # pooling: poolT[f, g] = sum_n h3[n, f] * (batch[n] == g)
              for bi in range(nblk):
                Mg = m_tp.tile([P, P], f32, tag="Mg", name="Mg")
                nc.vector.tensor_scalar(
                    out=Mg[:], in0=iota_f[:],
                    scalar1=batg_sb[:][:, bi:bi + 1], scalar2=None,
                    op0=AL.is_equal)
                nc.tensor.matmul(poolT[:],
                                 lhsT=h3_sb[:][:, bi * H:(bi + 1) * H],
                                 rhs=Mg[:], start=(bi == 0),
                                 stop=(bi == nblk - 1))
              poolT_sb = s_tp.tile([H, P], f32, tag="poolTs")
              nc.vector.tensor_copy(out=poolT_sb[:], in_=poolT[:])
              nc.sync.dma_start(prd_in[:], poolT_sb[:])
              nc.gpsimd.collective_compute(
                  "AllReduce", AL.add, replica_groups=RG,
                  ins=[prd_in.opt()], outs=[prd_out.opt()])
              poolF = s_tp.tile([H, P], f32, tag="poolF")
              nc.sync.dma_start(poolF[:], prd_out[:])
              fin = fin_ps.tile([P, C], f32, tag="fin")
              nc.tensor.matmul(fin[:], lhsT=poolF[:], rhs=wl_sb[:],
                               start=True, stop=True)
              outf = e_tp.tile([P, C], f32, tag="outf")
              nc.vector.tensor_scalar(out=outf[:], in0=fin[:],
                                      scalar1=cinv_sb[:], scalar2=None,
                                      op0=AL.mult)
              outf2 = e_tp.tile([P, C], f32, tag="outf2")
              nc.vector.tensor_tensor(out=outf2[:], in0=outf[:],
                                      in1=biasl_sb[:], op=AL.add)
              nc.sync.dma_start(out_d.ap()[:, :], outf2[:][:G, :])

    nc.compile()
    return nc


def make_in_maps(pp, weights, G, n_cores=NCORES):
    W1, b1, W2, b2, W3, b3, Wl, bl, counts = weights
    H = pp["H"]
    C = np.asarray(Wl).shape[1]
    bf = ml_dtypes.bfloat16
    iota_row = np.arange(P, dtype=np.float32)
    iota_bf = np.ascontiguousarray(np.broadcast_to(iota_row, (P, P))).astype(bf)
    iota_f = np.ascontiguousarray(np.broadcast_to(iota_row, (P, P)))
    cinv = np.ones((P, 1), np.float32)
    cinv[:G, 0] = 1.0 / np.maximum(counts, 1.0)
    shared = {
        "iota_bf": iota_bf, "iota_f": iota_f,
        "ident": np.eye(P, dtype=np.float32).astype(bf),
        **{f"x0t{q}": pp["x0t"][q] for q in range(NWIN)},
        "w0": np.asarray(W1, np.float32), "w1": np.asarray(W2, np.float32),
        "w2": np.asarray(W3, np.float32),
        "bias0": np.ascontiguousarray(np.broadcast_to(b1, (P, H))).astype(np.float32),
        "bias1": np.ascontiguousarray(np.broadcast_to(b2, (P, H))).astype(np.float32),
        "bias2": np.ascontiguousarray(np.broadcast_to(b3, (P, H))).astype(np.float32),
        "wl": np.asarray(Wl, np.float32),
        "biasl": np.ascontiguousarray(np.broadcast_to(bl, (P, C))).astype(np.float32),
        "cinv": cinv,
    }
    maps = []
    for c in range(n_cores):
        m = dict(shared)
        m["eidx"] = pp["eidx16"][c]
        m["edst"] = pp["edst2"][c].astype(bf)
        m["xp"] = pp["xp"][c]
        m["dinv"] = pp["dinv"][c]
        m["batg"] = pp["batg"][c]
        maps.append(m)
    return maps


LAST_RESULT = None
LAST_NC = None
LAST_IN_MAPS = None
LAST_BUILD = None


def kernel(x, edge_index, batch, W1, b1, W2, b2, W3, b3, Wl, bl, **run_kwargs):
    """Full-input entry point. Shards across 8 cores, runs on HW, gathers."""
    global LAST_RESULT, LAST_NC, LAST_IN_MAPS, LAST_BUILD
    from concourse.bass_utils import run_bass_kernel_spmd

    x = np.asarray(x, np.float32)
    edge_index = np.asarray(edge_index)
    batch = np.asarray(batch)
    G = G_FULL
    C = np.asarray(Wl).shape[1]

    pp = preprocess(x, edge_index, batch)
    counts = np.bincount(batch.astype(np.int64), minlength=G).astype(np.float32)
    nc = build_nc(pp, G, C)
    in_maps = make_in_maps(pp, (W1, b1, W2, b2, W3, b3, Wl, bl, counts), G)
    res = run_bass_kernel_spmd(nc, in_maps, core_ids=list(range(NCORES)),
                               **run_kwargs)
    LAST_RESULT, LAST_NC, LAST_IN_MAPS = res, nc, in_maps
    LAST_BUILD = dict(pp=pp, G=G, C=C)
    return res.results[0]["out"].astype(np.float32)


# revision 32
# speedup vs baseline: 1.0721x; 1.0211x over previous
"""3-layer GCN + global mean pool + linear head on 8 Trainium2 NeuronCores.

Strategy (dst-sharded message passing, v2):
  - GCN normalization factorizes: norm_e = dinv[src]*dinv[dst], so each conv
    layer is  h' = relu( dinv * ((Adj+I) @ (dinv * h)) @ W + b ).  Only pure
    row gather + segment-sum on device; diagonal scalings are per-node ops.
  - Nodes (and their in-edges, self-loops appended) are sharded across the 8
    cores by contiguous dst ranges.
  - The h~ table is quartered BY BLOCK RANGE (window w = block quarter q of
    every core's slice).  Each quarter is AllGathered separately as soon as
    the previous layer finishes that quarter's blocks, so collectives hide
    under compute.  Gather windows == quarters; int16 indices are relative
    to the window table.
  - Edge stream per core: for sg (7 dst blocks): for w (rotated by sg so the
    first gathers of a layer spread across all four AllGathers): edges of
    the sg's blocks in window w, concatenated unpadded, group padded to 128
    tokens UNIFORMLY across cores (max-core length).  The h~ tables are
    fp8e4 (rows: 64B payload + 192B pad at the mandatory 256B stride);
    dma_gather fetches just the 64B payloads (elem 64 x fp8, stride 256B),
    <=1024 tokens per call, rotated over 4 SWDGE queues.
  - Segment-sum on the TensorEngine in fp8 DoubleRow mode: chunk PAIRS per
    instruction, aggT[64f,128d] += [2x128e,64f].T @ M[2x128e,128d], M built
    by a broadcast is_equal of edst2 (per-(block,window,chunk) dst_rel or
    -1, fp8 out) against an iota tile.  Chunk spans are uniform across cores (min/max
    over cores); out-of-block tokens carry -1 and contribute zero.
  - The layer weight applies after aggregation, then dinv/bias/relu DVE ops.
  - Mean-pool uses the same one-hot matmul against graph ids; partials are
    AllReduced; the head matmul finishes on every core.
"""
import numpy as np
import ml_dtypes

P = 128
NCORES = 8
NWIN = 4          # block-quarter windows
TMAX = 1024       # max tokens per dma_gather call (SWDGE ring limit)
SGBLK = 7         # dst blocks per super-group (msgs buffer granularity)

# Full-size problem dims (nn_GCN_13881334300836)
N_FULL, E_FULL, D_FULL, C_FULL, G_FULL = 100_000, 1_250_000, 64, 10, 128


# --------------------------------------------------------------------------
# Host preprocessing
# --------------------------------------------------------------------------

def preprocess(x, edge_index, batch, n_cores=NCORES):
    """Shard nodes/edges; build window-rotated, group-padded gather streams
    with cross-core-uniform chunk spans.

    Table row for node n (c=n//npc, local=n-c*npc, p=local%P, b=local//P,
    q=quarter(b)): window q, row (c*P + p)*nbq[q] + (b - b0[q]).
    """
    N, H = x.shape
    assert N % n_cores == 0
    npc = N // n_cores
    nblk = (npc + P - 1) // P
    npad = nblk * P

    # block quarters (windows)
    base, rem = divmod(nblk, NWIN)
    nbq = [base + (1 if q < rem else 0) for q in range(NWIN)]
    b0q = np.cumsum([0] + nbq)          # len NWIN+1
    wrows = [n_cores * P * nbq[q] for q in range(NWIN)]
    assert max(wrows) <= 32768

    sgblk = SGBLK
    if nblk % sgblk != 0:
        sgblk = next((g for g in (7, 8, 6, 5, 4, 9, 10, 3, 2) if nblk % g == 0),
                     nblk)
    nsg = nblk // sgblk

    ei = edge_index.astype(np.int64)
    # degrees include self-loops (PyG gcn_norm), but self-loop messages are
    # local (identity matmul on-device) -- exclude them from the stream
    src_all = ei[0]
    dst_all = ei[1]

    deg = (np.bincount(dst_all, minlength=N) + 1).astype(np.float32)
    dinv = (1.0 / np.sqrt(np.maximum(deg, 1.0))).astype(np.float32)

    # source -> (window, idx16)
    core_of = src_all // npc
    local = src_all - core_of * npc
    p_of = local % P
    b_of = local // P
    q_of = np.searchsorted(b0q, b_of, side="right") - 1
    nbq_arr = np.asarray(nbq)
    idx_of = (core_of * P + p_of) * nbq_arr[q_of] + (b_of - b0q[q_of])

    # per-core sorted edge lists: key (dst block, window), stable
    core_edges = []   # c -> (off, s16, dr)
    cnt_all = np.zeros((n_cores, nblk, NWIN), np.int64)
    for c in range(n_cores):
        lo = c * npc
        m = (dst_all >= lo) & (dst_all < lo + npc)
        s16 = idx_of[m].astype(np.int16)
        w = q_of[m]
        d = dst_all[m] - lo
        db, dr = d // P, (d % P).astype(np.float32)
        key = db * NWIN + w
        order = np.argsort(key, kind="stable")
        s16, dr, key = s16[order], dr[order], key[order]
        cnt = np.bincount(key, minlength=nblk * NWIN).reshape(nblk, NWIN)
        cnt_all[c] = cnt
        off = np.zeros(nblk * NWIN + 1, np.int64)
        np.cumsum(cnt.ravel(), out=off[1:])
        core_edges.append((off, s16, dr))

    # uniform group lengths: for (sg, w): tokens = max_c sum_b cnt, pad to 128
    grp_len = np.zeros((nsg, NWIN), np.int64)
    for sg in range(nsg):
        bs = list(range(sg * sgblk, (sg + 1) * sgblk))
        for w in range(NWIN):
            mx = max(int(cnt_all[c, bs, w].sum()) for c in range(n_cores))
            grp_len[sg, w] = -(-max(mx, 1) // P) * P

    # stream layout: for sg: for w in rot(sg): group
    rot = [[(sg + j) % NWIN for j in range(NWIN)] for sg in range(nsg)]
    grp_tok0 = np.zeros((nsg, NWIN), np.int64)   # token start of (sg, w)
    sg_tok0 = []
    sg_w_ranges = []   # sg -> [(w, tok0, tok1)] in rotated order
    pos = 0
    for sg in range(nsg):
        sg_tok0.append(pos)
        rngs = []
        for w in rot[sg]:
            grp_tok0[sg, w] = pos
            rngs.append((w, pos, pos + int(grp_len[sg, w])))
            pos += int(grp_len[sg, w])
        sg_w_ranges.append(rngs)
    ntok = pos
    nchunk = ntok // P

    # per-(b, w) uniform chunk spans + matmul metadata
    spans = {}          # (b, w) -> (c0, c1)  global chunk ids
    ncol2 = 0
    col2_of = {}        # (b, w) -> starting edst2 column
    for sg in range(nsg):
        for b in range(sg * sgblk, (sg + 1) * sgblk):
            for w in range(NWIN):
                g0 = int(grp_tok0[sg, w])
                t0s, t1s = [], []
                for c in range(n_cores):
                    pre = int(cnt_all[c, sg * sgblk:b, w].sum())
                    cn = int(cnt_all[c, b, w])
                    t0s.append(g0 + pre)
                    t1s.append(g0 + pre + cn)
                c0 = min(t0s) // P
                c1 = -(-max(t1s) // P)
                c1 = max(c1, c0 + 1)
                spans[(b, w)] = (int(c0), int(c1))
                col2_of[(b, w)] = ncol2
                ncol2 += int(c1 - c0)

    # build per-core streams + edst2
    eidx16 = np.zeros((n_cores, 16, ntok // 16), np.int16)
    edst2 = np.full((n_cores, P, ncol2), -1.0, np.float32)
    for c in range(n_cores):
        off, s16, dr = core_edges[c]
        stream = np.zeros(ntok, np.int16)
        drel = np.full(ntok, -1.0, np.float32)   # dst_rel per token
        bof = np.full(ntok, -1, np.int64)        # owning block per token
        for sg in range(nsg):
            for w in range(NWIN):
                t = int(grp_tok0[sg, w])
                for b in range(sg * sgblk, (sg + 1) * sgblk):
                    k = b * NWIN + w
                    sl = slice(off[k], off[k + 1])
                    n_e = int(off[k + 1] - off[k])
                    stream[t:t + n_e] = s16[sl]
                    drel[t:t + n_e] = dr[sl]
                    bof[t:t + n_e] = b
                    t += n_e
        eidx16[c] = stream.reshape(ntok // 16, 16).T
        for (b, w), (c0, c1) in spans.items():
            cw = col2_of[(b, w)]
            seg_d = drel[c0 * P:c1 * P].copy()
            seg_b = bof[c0 * P:c1 * P]
            seg_d[seg_b != b] = -1.0
            edst2[c][:, cw:cw + (c1 - c0)] = seg_d.reshape(c1 - c0, P).T

    dinv_pc = np.zeros((n_cores, P, nblk), np.float32)
    bat_pc = np.full((n_cores, P, nblk), -1.0, np.float32)
    xp_pc = np.zeros((n_cores, P, nblk * H), np.float32)
    xf = np.asarray(x, np.float32)
    for c in range(n_cores):
        dv = np.zeros(npad, np.float32)
        dv[:npc] = dinv[c * npc:(c + 1) * npc]
        dinv_pc[c] = dv.reshape(nblk, P).T
        bt = np.full(npad, -1.0, np.float32)
        bt[:npc] = batch[c * npc:(c + 1) * npc].astype(np.float32)
        bat_pc[c] = bt.reshape(nblk, P).T
        xp = np.zeros((npad, H), np.float32)
        xp[:npc] = xf[c * npc:(c + 1) * npc]
        xp_pc[c] = xp.reshape(nblk, P, H).transpose(1, 0, 2).reshape(P, nblk * H)

    # layer-0 gather tables: h~0 = dinv * x for ALL nodes, window-laid-out
    # (host-computable, so layer 0 needs no AllGather at runtime).
    # fp8e4 rows: [64 B payload | 192 B pad] at the required 256 B stride.
    import concourse.mybir as _mybir
    f8d = _mybir.dt.np(_mybir.dt.float8e4)
    ht0 = (xf * dinv[:, None]).astype(f8d)
    n_all = np.arange(N)
    cn = n_all // npc
    ln = n_all - cn * npc
    pn, bn = ln % P, ln // P
    qn = np.searchsorted(b0q, bn, side="right") - 1
    rn = (cn * P + pn) * nbq_arr[qn] + (bn - b0q[qn])
    x0t = []
    for q in range(NWIN):
        Tq = np.zeros((n_cores * P * nbq[q], 2 * P), f8d)
        mq = qn == q
        Tq[rn[mq], :H] = ht0[n_all[mq]]
        x0t.append(Tq.reshape(n_cores * P, nbq[q] * 2 * P))

    return dict(eidx16=eidx16, edst2=edst2, dinv=dinv_pc, batg=bat_pc,
                xp=xp_pc, x0t=x0t, npc=npc, nblk=nblk, nsg=nsg, sgblk=sgblk,
                ntok=ntok, nchunk=nchunk, ncol2=ncol2, spans=spans,
                col2_of=col2_of, sg_tok0=sg_tok0, sg_w_ranges=sg_w_ranges,
                grp_tok0=grp_tok0, nbq=nbq, b0q=b0q, wrows=wrows, H=H)


# --------------------------------------------------------------------------
# Device kernel builder
# --------------------------------------------------------------------------

def dma_gather_any(gp, out_ap, in_ap, idxs_ap, num_idxs, num_idxs_reg,
                   elem_size, elem_step, single_packet=True, queue_num=0):
    """dma_gather with elem_size_bytes not restricted to %256 (non-transpose
    HBM-source path only; stride (elem_step) must still be a 256B multiple).
    Mirrors bass.GpSimd.dma_gather minus the transpose-only elem assert."""
    import concourse.mybir as mybir
    from concourse import ap_utils
    from concourse.bass import exact_div

    gp._assert_queue_num(queue_num)
    assert idxs_ap.dtype == mybir.dt.int16
    assert in_ap.dtype == out_ap.dtype
    elem_size_bytes = elem_size * mybir.dt.size(in_ap.dtype)
    assert elem_size_bytes > 0
    assert in_ap.space.name == "DRAM"
    assert idxs_ap.space.name == "SBUF"
    assert out_ap.space.name == "SBUF"
    assert ap_utils.ap_is_contiguous(out_ap.ap[1:])
    assert ap_utils.ap_is_contiguous(idxs_ap.ap[1:])
    assert in_ap.ap[-1][1] == out_ap.ap[-1][1] == elem_size
    assert out_ap.ap[0][1] * out_ap.ap[1][1] % 128 == 0
    assert in_ap.ap[0][0] == elem_step
    stride_bytes = elem_step * mybir.dt.size(in_ap.dtype)
    stride_bytes_256 = exact_div(stride_bytes, 256)
    assert stride_bytes_256 < 256
    _in_ap = gp.lower_ap_dma(in_ap, for_custom_bir_dma=True)
    _idxs_ap = gp.lower_ap(idxs_ap)
    _out_ap = gp.lower_ap(out_ap)
    return gp.add_instruction(
        mybir.InstDMAGatherAnt(
            name=gp.bass.get_next_instruction_name(),
            ins=[*_in_ap, _idxs_ap,
                 gp.lower_val_access(gp.to_reg(num_idxs_reg))],
            outs=[_out_ap],
            transpose=False,
            num_idxs=num_idxs,
            elem_size=elem_size,
            stride_bytes_256=stride_bytes_256,
            gen_mode=0,
            single_packet=single_packet,
            queue_num=queue_num,
            sbuf_tokens_per_rank=0,
            sbuf_free_dim_per_rank=0,
            sbuf_free_dim_pad_per_rank=0,
            sbuf_byte_offset=0,
        ))


def build_nc(pp, G, C, n_cores=NCORES, repeat=1, skip=frozenset(),
             nq=4, tmax=TMAX, single_packet=True):
    """Build the Bass program (shared SPMD across n_cores).

    repeat>1 re-runs the whole forward pass that many times inside one NEFF
    (delta-method HW timing only).  skip: timing-experiment knob."""
    import concourse.bacc as bacc
    import concourse.mybir as mybir
    import concourse.tile as tile
    from contextlib import ExitStack

    H = pp["H"]
    nblk, nsg, sgblk = pp["nblk"], pp["nsg"], pp["sgblk"]
    ntok, nchunk, ncol2 = pp["ntok"], pp["nchunk"], pp["ncol2"]
    spans, col2_of = pp["spans"], pp["col2_of"]
    sg_tok0, sg_w_ranges = pp["sg_tok0"], pp["sg_w_ranges"]
    nbq, b0q = pp["nbq"], pp["b0q"]
    RG = [list(range(n_cores))]
    EL = P  # padded table row width in bf16 elements (256B rows)

    f32, bf16 = mybir.dt.float32, mybir.dt.bfloat16
    i16 = mybir.dt.int16
    AL = mybir.AluOpType

    nc = bacc.Bacc("TRN2", target_bir_lowering=False, debug=False,
                   enable_asserts=False, num_devices=n_cores,
                   num_swdge_queues=nq)

    eidx_d = nc.dram_tensor("eidx", [16, ntok // 16], i16, kind="ExternalInput")
    edst_d = nc.dram_tensor("edst", [P, ncol2], bf16, kind="ExternalInput")
    xp_d = nc.dram_tensor("xp", [P, nblk * H], f32, kind="ExternalInput")
    dinv_d = nc.dram_tensor("dinv", [P, nblk], f32, kind="ExternalInput")
    batg_d = nc.dram_tensor("batg", [P, nblk], f32, kind="ExternalInput")
    iota_bf_d = nc.dram_tensor("iota_bf", [P, P], bf16, kind="ExternalInput")
    ident_d = nc.dram_tensor("ident", [P, P], bf16, kind="ExternalInput")
    iota_f_d = nc.dram_tensor("iota_f", [P, P], f32, kind="ExternalInput")
    w_d = [nc.dram_tensor(f"w{l}", [H, H], f32, kind="ExternalInput")
           for l in range(3)]
    bias_d = [nc.dram_tensor(f"bias{l}", [P, H], f32, kind="ExternalInput")
              for l in range(3)]
    wl_d = nc.dram_tensor("wl", [H, C], f32, kind="ExternalInput")
    biasl_d = nc.dram_tensor("biasl", [P, C], f32, kind="ExternalInput")
    cinv_d = nc.dram_tensor("cinv", [P, 1], f32, kind="ExternalInput")
    f8 = mybir.dt.float8e4
    EL8 = 2 * P   # fp8 elems per 256B table row
    x0t_d = [nc.dram_tensor(f"x0t{q}", [n_cores * P, nbq[q] * EL8], f8,
                            kind="ExternalInput") for q in range(NWIN)]
    out_d = nc.dram_tensor("out", [G, C], f32, kind="ExternalOutput")

    with tile.TileContext(nc) as tc:
        with ExitStack() as ctx:
            const = ctx.enter_context(tc.tile_pool(name="const", bufs=1))
            msgs_tp = ctx.enter_context(tc.tile_pool(name="msgs", bufs=2))
            m_tp = ctx.enter_context(tc.tile_pool(name="mb", bufs=3))
            s_tp = ctx.enter_context(tc.tile_pool(name="st", bufs=3))
            e_tp = ctx.enter_context(tc.tile_pool(name="ep", bufs=4))
            agg_ps = ctx.enter_context(tc.tile_pool(name="aggp", bufs=4,
                                                    space="PSUM"))
            out_ps = ctx.enter_context(tc.tile_pool(name="outp", bufs=2,
                                                    space="PSUM"))
            fin_ps = ctx.enter_context(tc.tile_pool(name="finp", bufs=1,
                                                    space="PSUM"))
            dram = ctx.enter_context(tc.tile_pool(name="dram", bufs=1,
                                                  space="DRAM"))

            eidx_sb = const.tile([128, ntok // 16], i16)
            edst_sb = const.tile([P, ncol2], bf16)
            iota_bf = const.tile([P, P], bf16)
            iota_f = const.tile([P, P], f32)
            ident_bf = const.tile([P, P], bf16)
            dinv_sb = const.tile([P, nblk], f32)
            batg_sb = const.tile([P, nblk], f32)
            w_sb = [const.tile([H, H], f32, tag=f"w{l}", name=f"w{l}_sb")
                    for l in range(3)]
            bias_sb = [const.tile([P, H], f32, tag=f"b{l}", name=f"b{l}_sb")
                       for l in range(3)]
            wl_sb = const.tile([H, C], f32)
            biasl_sb = const.tile([P, C], f32)
            cinv_sb = const.tile([P, 1], f32)
            ht_sb = const.tile([P, nblk, EL], bf16)   # h~ slice, 256B rows
            ht8_sb = const.tile([P, nblk, H], f8)     # fp8 h~ (64B payload)
            h3_sb = const.tile([P, nblk * H], f32)
            xp_sb = const.tile([P, nblk * H], f32)

            # idx tile: replicate the [16, S] wrap to all 8 partition groups
            for g8 in range(8):
                nc.sync.dma_start(eidx_sb[:][g8 * 16:(g8 + 1) * 16, :],
                                  eidx_d.ap())
            nc.sync.dma_start(edst_sb[:], edst_d.ap())
            nc.sync.dma_start(iota_bf[:], iota_bf_d.ap())
            nc.sync.dma_start(ident_bf[:], ident_d.ap())
            nc.sync.dma_start(iota_f[:], iota_f_d.ap())
            nc.sync.dma_start(dinv_sb[:], dinv_d.ap())
            nc.sync.dma_start(batg_sb[:], batg_d.ap())
            for l in range(3):
                nc.sync.dma_start(w_sb[l][:], w_d[l].ap())
                nc.sync.dma_start(bias_sb[l][:], bias_d[l].ap())
            nc.sync.dma_start(wl_sb[:], wl_d.ap())
            nc.sync.dma_start(biasl_sb[:], biasl_d.ap())
            nc.sync.dma_start(cinv_sb[:], cinv_d.ap())
            nc.sync.dma_start(xp_sb[:], xp_d.ap())
            # zero the padding feature columns of h~ once
            nc.vector.memset(ht_sb[:], 0.0)
            if "epi" in skip:
                nc.vector.memset(h3_sb[:], 0.0)

            # per-quarter staging + per-(repeat, layer, quarter) shared outs
            in_cc_q = [dram.tile([P, nbq[q] * EL8], f8, tag=f"incc{q}",
                                 name=f"incc{q}") for q in range(NWIN)]
            hfull_rlq = [[[dram.tile([n_cores * P, nbq[q] * EL8], f8,
                                     addr_space="Shared",
                                     tag=f"hf{r}_{l}_{q}",
                                     name=f"hf{r}_{l}_{q}")
                           for q in range(NWIN)] for l in (1, 2)]
                         for r in range(repeat)]
            prd_in = dram.tile([H, P], f32)
            prd_out_r = [dram.tile([H, P], f32, addr_space="Shared",
                                   tag=f"prd_out_{r}", name=f"prd_out_{r}")
                         for r in range(repeat)]

            def issue_ag(r, l, q):
                """Stage quarter q of ht8 (fp8 payload halves of the 256B
                rows) and AllGather it for layer l (hfull index l-1)."""
                nc.sync.dma_start(
                    in_cc_q[q][:].rearrange("p (b e) -> p b e", e=EL8)
                        [:, :, 0:H],
                    ht8_sb[:][:, int(b0q[q]):int(b0q[q + 1]), :])
                if "ag" not in skip:
                    nc.gpsimd.collective_compute(
                        "AllGather", AL.bypass, replica_groups=RG,
                        ins=[in_cc_q[q].opt()],
                        outs=[hfull_rlq[r][l - 1][q].opt()])

            qend = {int(b0q[q + 1]) - 1: q for q in range(NWIN)}

            for _rep in range(repeat):
              hfull = hfull_rlq[_rep]
              prd_out = prd_out_r[_rep]

              # layer-1 input: h~ = dinv * x (bf16) -- self-loop source only;
              # layer 0's gather tables are host inputs (no AllGather)
              for bi in range(nblk):
                if "hscale" not in skip:
                    nc.vector.tensor_scalar(
                        out=ht_sb[:][:, bi, 0:H],
                        in0=xp_sb[:][:, bi * H:(bi + 1) * H],
                        scalar1=dinv_sb[:][:, bi:bi + 1], scalar2=None,
                        op0=AL.mult)

              poolT = fin_ps.tile([H, P], f32, tag="poolT")
              for l in range(3):
                last = l == 2
                if l == 0:
                    gat = [x0t_d[q].ap()
                               .rearrange("p (b e) -> (p b) e", e=EL8)
                           for q in range(NWIN)]
                else:
                    gat = [hfull[l - 1][q][:]
                               .rearrange("p (b e) -> (p b) e", e=EL8)
                           for q in range(NWIN)]

                call_no = 0
                for sg in range(nsg):
                    tok0 = sg_tok0[sg]
                    sg_ntok = sg_w_ranges[sg][-1][2] - tok0
                    msgs = msgs_tp.tile([P, sg_ntok // P, H], f8,
                                        tag="msgs", name="msgs")
                    for (ww, t0, t1) in sg_w_ranges[sg]:
                        t = t0
                        while t < t1:
                            tc_ = min(tmax, t1 - t)
                            if "gather" in skip:
                                t += tc_
                                continue
                            dma_gather_any(
                                nc.gpsimd,
                                out_ap=msgs[:][:, (t - tok0) // P:
                                               (t - tok0 + tc_) // P, :],
                                in_ap=gat[ww][:, 0:H],
                                idxs_ap=eidx_sb[:][:, t // 16:(t + tc_) // 16],
                                num_idxs=tc_, num_idxs_reg=tc_,
                                elem_size=H, elem_step=EL8,
                                single_packet=single_packet,
                                queue_num=call_no % nq)
                            call_no += 1
                            t += tc_
                    for bi in range(sg * sgblk, (sg + 1) * sgblk):
                        aggT = agg_ps.tile([H, P], f32, tag="agg", name="agg")
                        kbt = sum(spans[(bi, w)][1] - spans[(bi, w)][0]
                                  for w in range(NWIN))
                        nmm = 1 + kbt
                        if "mm" not in skip:
                            # self-loop: aggT[f, d] += ht[d, b, f]
                            nc.tensor.matmul(
                                aggT[:], lhsT=ht_sb[:][:, bi, 0:H],
                                rhs=ident_bf[:], start=True, stop=False)
                        imm = 1
                        cw0 = col2_of[(bi, 0)]
                        MB = m_tp.tile([P, kbt * P], f8, tag="MB",
                                       name="MB")
                        if "mb" not in skip:
                            nc.vector.tensor_tensor(
                                out=MB[:].rearrange("p (c q) -> p c q", q=P),
                                in0=edst_sb[:][:, cw0:cw0 + kbt]
                                    .to_broadcast([P, kbt, P]),
                                in1=iota_bf[:][:, None, :]
                                    .to_broadcast([P, kbt, P]),
                                op=AL.is_equal)
                        MBr = MB[:].rearrange("p (c q) -> p c q", q=P)
                        for w in range(NWIN):
                            c0, c1 = spans[(bi, w)]
                            kb = c1 - c0
                            joff = col2_of[(bi, w)] - cw0
                            j = 0
                            while j < kb:
                                mc = c0 + j - tok0 // P
                                pair = j + 1 < kb
                                nj = 2 if pair else 1
                                if "mm" in skip:
                                    imm += nj
                                    j += nj
                                    continue
                                if pair:
                                    nc.tensor.matmul(
                                        aggT[:],
                                        lhsT=msgs[:][:, mc:mc + 2, :],
                                        rhs=MBr[:, joff + j:joff + j + 2, :],
                                        start=(imm == 0),
                                        stop=(imm + 2 == nmm),
                                        perf_mode=(
                                            mybir.MatmulPerfMode.DoubleRow),
                                    )
                                else:
                                    nc.tensor.matmul(
                                        aggT[:],
                                        lhsT=msgs[:][:, mc, :],
                                        rhs=MBr[:, joff + j, :],
                                        start=(imm == 0),
                                        stop=(imm + 1 == nmm))
                                imm += nj
                                j += nj
                        if "epi" in skip:
                            continue
                        sT = s_tp.tile([H, P], f32, tag="sT", name="sT")
                        nc.scalar.copy(out=sT[:], in_=aggT[:])
                        outb = out_ps.tile([P, H], f32, tag="outb",
                                           name="outb")
                        nc.tensor.matmul(outb[:], lhsT=sT[:], rhs=w_sb[l][:],
                                         start=True, stop=True)
                        dcol = dinv_sb[:][:, bi:bi + 1]
                        t1_ = e_tp.tile([P, H], f32, tag="t1", name="t1")
                        nc.vector.tensor_scalar(
                            out=t1_[:], in0=outb[:], scalar1=dcol,
                            scalar2=None, op0=AL.mult)
                        if not last:
                            t2 = e_tp.tile([P, H], f32, tag="t2", name="t2")
                            nc.vector.tensor_tensor(
                                out=t2[:], in0=t1_[:], in1=bias_sb[l][:],
                                op=AL.add)
                            nc.vector.tensor_scalar(
                                out=ht_sb[:][:, bi, 0:H], in0=t2[:],
                                scalar1=0.0, scalar2=dcol,
                                op0=AL.max, op1=AL.mult)
                            nc.vector.tensor_scalar(
                                out=ht8_sb[:][:, bi, :], in0=t2[:],
                                scalar1=0.0, scalar2=dcol,
                                op0=AL.max, op1=AL.mult)
                            if bi in qend:
                                issue_ag(_rep, l + 1, qend[bi])
                        else:
                            nc.vector.tensor_tensor(
                                out=h3_sb[:][:, bi * H:(bi + 1) * H],
                                in0=t1_[:], in1=bias_sb[l][:], op=AL.add)

              # pooling: poolT[f, g] = sum_n h3[n, f] * (batch[n] == g)
              for bi in range(nblk):
                Mg = m_tp.tile([P, P], f32, tag="Mg", name="Mg")
                nc.vector.tensor_scalar(
                    out=Mg[:], in0=iota_f[:],
                    scalar1=batg_sb[:][:, bi:bi + 1], scalar2=None,
                    op0=AL.is_equal)
                nc.tensor.matmul(poolT[:],
                                 lhsT=h3_sb[:][:, bi * H:(bi + 1) * H],
                                 rhs=Mg[:], start=(bi == 0),
                                 stop=(bi == nblk - 1))
              poolT_sb = s_tp.tile([H, P], f32, tag="poolTs")
              nc.vector.tensor_copy(out=poolT_sb[:], in_=poolT[:])
              nc.sync.dma_start(prd_in[:], poolT_sb[:])
              nc.gpsimd.collective_compute(
                  "AllReduce", AL.add, replica_groups=RG,
                  ins=[prd_in.opt()], outs=[prd_out.opt()])
              poolF = s_tp.tile([H, P], f32, tag="poolF")
              nc.sync.dma_start(poolF[:], prd_out[:])
              fin = fin_ps.tile([P, C], f32, tag="fin")
              nc.tensor.matmul(fin[:], lhsT=poolF[:], rhs=wl_sb[:],
                               start=True, stop=True)
              outf = e_tp.tile([P, C], f32, tag="outf")
              nc.vector.tensor_scalar(out=outf[:], in0=fin[:],
                                      scalar1=cinv_sb[:], scalar2=None,
                                      op0=AL.mult)
              outf2 = e_tp.tile([P, C], f32, tag="outf2")
              nc.vector.tensor_tensor(out=outf2[:], in0=outf[:],
                                      in1=biasl_sb[:], op=AL.add)
              nc.sync.dma_start(out_d.ap()[:, :], outf2[:][:G, :])

    nc.compile()
    return nc


def make_in_maps(pp, weights, G, n_cores=NCORES):
    W1, b1, W2, b2, W3, b3, Wl, bl, counts = weights
    H = pp["H"]
    C = np.asarray(Wl).shape[1]
    bf = ml_dtypes.bfloat16
    iota_row = np.arange(P, dtype=np.float32)
    iota_bf = np.ascontiguousarray(np.broadcast_to(iota_row, (P, P))).astype(bf)
    iota_f = np.ascontiguousarray(np.broadcast_to(iota_row, (P, P)))
    cinv = np.ones((P, 1), np.float32)
    cinv[:G, 0] = 1.0 / np.maximum(counts, 1.0)
    shared = {
        "iota_bf": iota_bf, "iota_f": iota_f,
        "ident": np.eye(P, dtype=np.float32).astype(bf),
        **{f"x0t{q}": pp["x0t"][q] for q in range(NWIN)},
        "w0": np.asarray(W1, np.float32), "w1": np.asarray(W2, np.float32),
        "w2": np.asarray(W3, np.float32),
        "bias0": np.ascontiguousarray(np.broadcast_to(b1, (P, H))).astype(np.float32),
        "bias1": np.ascontiguousarray(np.broadcast_to(b2, (P, H))).astype(np.float32),
        "bias2": np.ascontiguousarray(np.broadcast_to(b3, (P, H))).astype(np.float32),
        "wl": np.asarray(Wl, np.float32),
        "biasl": np.ascontiguousarray(np.broadcast_to(bl, (P, C))).astype(np.float32),
        "cinv": cinv,
    }
    maps = []
    for c in range(n_cores):
        m = dict(shared)
        m["eidx"] = pp["eidx16"][c]
        m["edst"] = pp["edst2"][c].astype(bf)
        m["xp"] = pp["xp"][c]
        m["dinv"] = pp["dinv"][c]
        m["batg"] = pp["batg"][c]
        maps.append(m)
    return maps


LAST_RESULT = None
LAST_NC = None
LAST_IN_MAPS = None
LAST_BUILD = None


def kernel(x, edge_index, batch, W1, b1, W2, b2, W3, b3, Wl, bl, **run_kwargs):
    """Full-input entry point. Shards across 8 cores, runs on HW, gathers."""
    global LAST_RESULT, LAST_NC, LAST_IN_MAPS, LAST_BUILD
    from concourse.bass_utils import run_bass_kernel_spmd

    x = np.asarray(x, np.float32)
    edge_index = np.asarray(edge_index)
    batch = np.asarray(batch)
    G = G_FULL
    C = np.asarray(Wl).shape[1]

    pp = preprocess(x, edge_index, batch)
    counts = np.bincount(batch.astype(np.int64), minlength=G).astype(np.float32)
    nc = build_nc(pp, G, C)
    in_maps = make_in_maps(pp, (W1, b1, W2, b2, W3, b3, Wl, bl, counts), G)
    res = run_bass_kernel_spmd(nc, in_maps, core_ids=list(range(NCORES)),
                               **run_kwargs)
    LAST_RESULT, LAST_NC, LAST_IN_MAPS = res, nc, in_maps
    LAST_BUILD = dict(pp=pp, G=G, C=C)
    return res.results[0]["out"].astype(np.float32)
